# revision 7
# baseline (speedup 1.0000x reference)
"""GCN graph classifier on 8 Trainium2 NeuronCores (Bass/Tile) — single launch.

v2 design (graph/data parallel per the sharding hint):
- Nodes split into 8 contiguous graph-aligned ranges; each core owns the
  destination side of every edge landing in its range, the pooling and the
  MLP head for its graphs.
- Both GCN layers, the inter-layer hidden-state exchange (DRAM AllGather),
  the mean-pool and the MLP head run in ONE device launch per kernel call.
- Tables live in DRAM in "slice layout" (row r2(n) = owner*W_SLOTS*128 +
  (n - n0[owner])), 256-byte bf16 rows; per-edge rows are fetched with
  dma_gather (int16 indices, 4 chunk windows of 32768 rows).
- Scatter-accumulate uses one-hot matmuls: per 128-edge block, a bf16
  one-hot built on DVE (iota == dst_rel, scaled by rsqrt(deg_src*deg_dst))
  feeds a PE matmul accumulating into a per-slot PSUM window. Blocks are
  packed without per-slot padding; a block whose edges straddle slot
  boundaries issues one matmul per touched slot.
- Self-loop terms use resident SBUF tiles (x / layer-1 hidden) with a
  diagonal one-hot of 1/deg, skipping ~25k gather descriptors per core.
- All per-core variation (indices, one-hot selectors, degrees) is input
  data; the compiled program is identical across cores (SPMD).

Self-contained: no imports from the problem directory.
"""
import time

import numpy as np

import concourse.bass as bass
import concourse.bacc as bacc
import concourse.mybir as mybir
import concourse.tile as tile

N_NODES = 100_000
N_EDGES = 1_200_000
N_GRAPHS = 512
HID = 64
NCORES = 8
P = 128
CHUNK_ROWS = 32_768
GSLOTS = 8                # slots per PSUM group
GMAX = 96                 # padded graphs per core
SENT = 30_000.0           # one-hot sentinel (bf16 -> 29952, never matches)

F32 = mybir.dt.float32
BF16 = mybir.dt.bfloat16
I16 = mybir.dt.int16
NPBF = mybir.dt.np(BF16)


# ---------------------------------------------------------------- host prep

def _prep(edge_index: np.ndarray, batch: np.ndarray):
    """Index-side preprocessing only: integer index manipulation derived from
    the graph structure plus integer degree counts (rsqrt happens on device)."""
    src = np.asarray(edge_index[0], dtype=np.int64)
    dst = np.asarray(edge_index[1], dtype=np.int64)
    batch = np.asarray(batch, dtype=np.int64)

    deg = np.bincount(dst, minlength=N_NODES) + 1  # +1 self-loop

    gptr = np.searchsorted(batch, np.arange(N_GRAPHS + 1))
    targets = (np.arange(1, NCORES) * N_NODES) // NCORES
    gsplit = np.searchsorted(gptr, targets)
    g0s = np.concatenate([[0], gsplit, [N_GRAPHS]])
    n0s = gptr[g0s]  # len 9
    nowns = np.diff(n0s)
    W_SLOTS = int(max(-(-n // P) for n in nowns))
    NTAB = NCORES * W_SLOTS * P
    NCHUNK = -(-NTAB // CHUNK_ROWS)
    NGRP = -(-W_SLOTS // GSLOTS)
    W_PAD = NGRP * GSLOTS
    assert max(g0s[c + 1] - g0s[c] for c in range(NCORES)) <= GMAX

    owner = np.searchsorted(n0s[1:], np.arange(N_NODES), side="right")
    r2 = owner * (W_SLOTS * P) + (np.arange(N_NODES) - n0s[owner])

    NCELL = NGRP * NCHUNK
    cores_raw = []
    cnt = np.zeros((NCORES, NCELL), dtype=np.int64)
    for c in range(NCORES):
        n0, n1 = int(n0s[c]), int(n0s[c + 1])
        m = (dst >= n0) & (dst < n1)
        es, ed = src[m], dst[m]
        slot = (ed - n0) >> 7
        grp = slot >> 3
        chunk = r2[es] >> 15
        o = np.lexsort((ed, slot, chunk, grp))
        es, ed, slot, grp, chunk = es[o], ed[o], slot[o], grp[o], chunk[o]
        cell = grp * NCHUNK + chunk
        cnt[c] = np.bincount(cell, minlength=NCELL)
        cores_raw.append((n0, n1, es, ed, slot, cell))

    K = -(-cnt.max(axis=0) // P)          # blocks per cell (0 stays 0)
    block_base = np.concatenate([[0], np.cumsum(K)[:-1]])
    NBLK = int(K.sum())
    NE = NBLK * P

    # calls: one gather per non-empty (group, chunk)
    calls = []  # (g, ch, blk0, nblk)
    for g in range(NGRP):
        for ch in range(NCHUNK):
            cell = g * NCHUNK + ch
            if K[cell] > 0:
                calls.append((g, ch, int(block_base[cell]), int(K[cell])))
    call_of_cell = {}
    for ci, (g, ch, b0, nb) in enumerate(calls):
        call_of_cell[g * NCHUNK + ch] = ci

    # per-core edge placement + per-block slot spans
    lo = np.full(NBLK, 1 << 30, dtype=np.int64)
    hi = np.full(NBLK, -1, dtype=np.int64)
    percore_edges = []
    for c in range(NCORES):
        n0, n1, es, ed, slot, cell = cores_raw[c]
        cell_start = np.concatenate([[0], np.cumsum(np.bincount(
            cell, minlength=NCELL))])[:-1]
        rank = np.arange(len(es)) - cell_start[cell]
        blk = block_base[cell] + (rank >> 7)
        pos = blk * P + (rank & 127)
        np.minimum.at(lo, blk, slot)
        np.maximum.at(hi, blk, slot)
        percore_edges.append((n0, es, ed, slot, blk, rank & 127))
    assert (hi >= 0).all()

    # mm schedule (shared across cores). PSUM accumulation groups to the same
    # bank must be contiguous in PE issue order, so emit slot-major: each
    # slot's self mm + edge mms form one contiguous start..stop group.
    mm_list = []   # (kind, g, s, blk, ci, kloc, col) col=dst_rel column
    NMME = 0
    for g in range(NGRP):
        per_slot = {s: [] for s in range(g * GSLOTS, (g + 1) * GSLOTS)}
        for ch in range(NCHUNK):
            cell = g * NCHUNK + ch
            for k in range(int(K[cell])):
                blk = int(block_base[cell]) + k
                for s in range(int(lo[blk]), int(hi[blk]) + 1):
                    per_slot[s].append((blk, call_of_cell[cell], k))
        for s in range(g * GSLOTS, (g + 1) * GSLOTS):
            mm_list.append(["self", g, s, -1, -1, -1, -1])
            for blk, ci, k in per_slot[s]:
                mm_list.append(["edge", g, s, blk, ci, k, NMME])
                NMME += 1
            mm_list[-1].append("stop")
    NMM = len(mm_list)

    # per-core data arrays
    rng_pad = np.random.default_rng(12345)
    percore = []
    for c in range(NCORES):
        n0, es, ed, slot, blk, wpos = percore_edges[c]

        idx_flat = rng_pad.integers(0, 2048, size=NE).astype(np.int16)
        idx_flat[blk * P + wpos] = (r2[es] & (CHUNK_ROWS - 1)).astype(np.int16)
        cols = NE // 16
        arr = np.zeros((16, cols), dtype=np.int16)
        j = np.arange(NE)
        arr[j % 16, j // 16] = idx_flat
        idx_packed = np.tile(arr, (8, 1))

        # dst_rel: one column per edge mm
        mme_of_blk_s = {}
        for mi, mm in enumerate(mm_list):
            if mm[0] == "edge":
                mme_of_blk_s[(mm[3], mm[2])] = mm[6]
        col = np.array([mme_of_blk_s[(b, s)] for b, s in zip(blk, slot)])
        dst_rel = np.full((P, NMME), SENT, dtype=np.float32)
        dst_rel[wpos, col] = (ed - n0 - slot * P).astype(np.float32)

        dsnp = np.ones((P, NBLK), dtype=np.float32)
        dsnp[wpos, blk] = (deg[es] * deg[ed]).astype(np.float32)

        nown = int(n0s[c + 1]) - n0
        ar = np.arange(nown)
        deg_own = np.ones((P, W_PAD), dtype=np.float32)
        deg_own[ar % P, ar >> 7] = deg[n0:n0 + nown].astype(np.float32)
        g_rel = np.full((P, W_PAD), SENT, dtype=np.float32)
        g_rel[ar % P, ar >> 7] = (batch[n0:n0 + nown] - g0s[c]).astype(
            np.float32)

        percore.append({
            "n0": n0, "nown": nown, "g0": int(g0s[c]), "g1": int(g0s[c + 1]),
            "idx_packed": idx_packed,
            "dst_rel": dst_rel,
            "dsnp": dsnp,
            "deg_own": deg_own,
            "g_rel": g_rel,
        })

    struct = {
        "W_SLOTS": W_SLOTS, "W_PAD": W_PAD, "NTAB": NTAB, "NGRP": NGRP,
        "NCHUNK": NCHUNK, "NBLK": NBLK, "NE": NE, "NMM": NMM, "NMME": NMME,
        "calls": calls, "mm_list": mm_list, "r2": r2, "n0s": n0s, "g0s": g0s,
    }
    return struct, percore


# ------------------------------------------------------------- bass program

def _build(struct, reps: int = 1, dump: bool = False,
           coll_reps: bool = True, l2_reps: bool = True, part: str = 'all'):
    W_SLOTS, W_PAD = struct["W_SLOTS"], struct["W_PAD"]
    NTAB, NGRP, NCHUNK = struct["NTAB"], struct["NGRP"], struct["NCHUNK"]
    NBLK, NE, NMME = struct["NBLK"], struct["NE"], struct["NMME"]
    calls, mm_list = struct["calls"], struct["mm_list"]

    nc = bacc.Bacc("TRN2", num_swdge_queues=4)
    xtab = nc.dram_tensor("xtab", (NTAB, P), BF16, kind="ExternalInput")
    idx_in = nc.dram_tensor("idx", (P, NE // 16), I16, kind="ExternalInput")
    dst_rel_in = nc.dram_tensor("dst_rel", (P, NMME), F32,
                                kind="ExternalInput")
    dsnp_in = nc.dram_tensor("dsnp", (P, NBLK), F32, kind="ExternalInput")
    deg_own_in = nc.dram_tensor("deg_own", (P, W_PAD), F32,
                                kind="ExternalInput")
    g_rel_in = nc.dram_tensor("g_rel", (P, W_PAD), F32, kind="ExternalInput")
    x_own_in = nc.dram_tensor("x_own", (P, 2 * W_PAD), BF16,
                              kind="ExternalInput")
    iota_in = nc.dram_tensor("iota", (P, P), BF16, kind="ExternalInput")
    self_rel_in = nc.dram_tensor("self_rel", (P, 1), F32,
                                 kind="ExternalInput")
    ident_in = nc.dram_tensor("ident", (P, P), F32, kind="ExternalInput")
    ident_bf_in = nc.dram_tensor("ident_bf", (P, P), BF16,
                                 kind="ExternalInput")
    ones_bf_in = nc.dram_tensor("ones_bf", (1, P), BF16, kind="ExternalInput")
    ones_f_in = nc.dram_tensor("ones_f", (1, P), F32, kind="ExternalInput")
    w1_in = nc.dram_tensor("W1", (2, HID), BF16, kind="ExternalInput")
    b1_in = nc.dram_tensor("b1", (1, HID), BF16, kind="ExternalInput")
    w2_in = nc.dram_tensor("W2", (HID, HID), BF16, kind="ExternalInput")
    b2_in = nc.dram_tensor("b2", (1, HID), BF16, kind="ExternalInput")
    wf1_in = nc.dram_tensor("Wf1", (HID, HID), F32, kind="ExternalInput")
    bf1_in = nc.dram_tensor("bf1", (1, HID), F32, kind="ExternalInput")
    wf2_in = nc.dram_tensor("Wf2", (HID, 4), F32, kind="ExternalInput")
    bf2_in = nc.dram_tensor("bf2", (1, 4), F32, kind="ExternalInput")

    h1slice = nc.dram_tensor("h1slice", (W_SLOTS * P, P), BF16)
    if dump:
        h1dump = nc.dram_tensor("h1dump", (W_SLOTS * P, HID), BF16,
                                kind="ExternalOutput")
    h1tab = nc.dram_tensor("h1tab", (NTAB, P), BF16)
    out_t = nc.dram_tensor("out", (GMAX, 4), F32, kind="ExternalOutput")

    with tile.TileContext(nc) as tc:
        with tc.tile_pool(name="const", bufs=1) as cpool, \
             tc.tile_pool(name="meta", bufs=1) as mpool, \
             tc.tile_pool(name="gat", bufs=3) as gpool, \
             tc.tile_pool(name="work", bufs=3) as wpool, \
             tc.tile_pool(name="oh", bufs=48) as ohpool, \
             tc.tile_pool(name="pacc", bufs=3, space="PSUM") as pacc, \
             tc.tile_pool(name="ptp", bufs=2, space="PSUM") as ptp, \
             tc.tile_pool(name="ppool", bufs=1, space="PSUM") as ppool:

            # ---- constants / metadata
            iota_t = cpool.tile([P, P], BF16)
            nc.sync.dma_start(out=iota_t[:], in_=iota_in[:])
            self_rel_t = cpool.tile([P, 1], F32)
            nc.sync.dma_start(out=self_rel_t[:], in_=self_rel_in[:])
            ident_t = cpool.tile([P, P], F32)
            nc.sync.dma_start(out=ident_t[:], in_=ident_in[:])
            ident_bf_t = cpool.tile([P, P], BF16)
            nc.sync.dma_start(out=ident_bf_t[:], in_=ident_bf_in[:])
            ones_bf_t = cpool.tile([1, P], BF16)
            nc.sync.dma_start(out=ones_bf_t[:], in_=ones_bf_in[:])
            ones_f_t = cpool.tile([1, P], F32)
            nc.sync.dma_start(out=ones_f_t[:], in_=ones_f_in[:])
            onecol_t = cpool.tile([P, 1], F32)
            nc.vector.memset(onecol_t[:], 1.0)
            w1_t = cpool.tile([2, HID], BF16)
            nc.sync.dma_start(out=w1_t[:], in_=w1_in[:])
            b1_t = cpool.tile([1, HID], BF16)
            nc.sync.dma_start(out=b1_t[:], in_=b1_in[:])
            w2_t = cpool.tile([HID, HID], BF16)
            nc.sync.dma_start(out=w2_t[:], in_=w2_in[:])
            b2_t = cpool.tile([1, HID], BF16)
            nc.sync.dma_start(out=b2_t[:], in_=b2_in[:])
            wf1_t = cpool.tile([HID, HID], F32)
            nc.sync.dma_start(out=wf1_t[:], in_=wf1_in[:])
            bf1_t = cpool.tile([1, HID], F32)
            nc.sync.dma_start(out=bf1_t[:], in_=bf1_in[:])
            wf2_t = cpool.tile([HID, 4], F32)
            nc.sync.dma_start(out=wf2_t[:], in_=wf2_in[:])
            bf2_t = cpool.tile([1, 4], F32)
            nc.sync.dma_start(out=bf2_t[:], in_=bf2_in[:])

            idx_t = mpool.tile([P, NE // 16], I16)
            nc.sync.dma_start(out=idx_t[:], in_=idx_in[:])
            dst_rel_t = mpool.tile([P, NMME], F32)
            nc.sync.dma_start(out=dst_rel_t[:], in_=dst_rel_in[:])
            g_rel_t = mpool.tile([P, W_PAD], F32)
            nc.sync.dma_start(out=g_rel_t[:], in_=g_rel_in[:])
            x_own_t = mpool.tile([P, 2 * W_PAD], BF16)
            nc.sync.dma_start(out=x_own_t[:], in_=x_own_in[:])

            # dsn = rsqrt(deg_src*deg_dst) in bf16
            dsnp_t = mpool.tile([P, NBLK], F32)
            nc.sync.dma_start(out=dsnp_t[:], in_=dsnp_in[:])
            dsnr_t = mpool.tile([P, NBLK], F32)
            nc.vector.reciprocal(out=dsnr_t[:], in_=dsnp_t[:])
            dsn_t = mpool.tile([P, NBLK], F32)
            nc.scalar.sqrt(out=dsn_t[:], in_=dsnr_t[:])

            # dinv2_own = 1/deg_own in bf16
            deg_own_t = mpool.tile([P, W_PAD], F32)
            nc.sync.dma_start(out=deg_own_t[:], in_=deg_own_in[:])
            dinv2_t = mpool.tile([P, W_PAD], F32)
            nc.vector.reciprocal(out=dinv2_t[:], in_=deg_own_t[:])

            # resident layer-1 hidden (own nodes), persists across layers
            h1own_t = mpool.tile([P, W_PAD * HID], BF16)

            dumtiles = None
            if part in ("mm", "ohonly", "tails"):
                maxnb = max(nb for (_, _, _, nb) in calls)
                dumtiles = {}
                for ch in range(NCHUNK):
                    dt_ = mpool.tile([P, maxnb, P], BF16, tag=f"dum{ch}")
                    nc.vector.memset(dt_[:], 0.125)
                    dumtiles[ch] = dt_

            pool_ps = ppool.tile([GMAX, HID + 1], F32, space="PSUM")

            for rep in range(reps):
                for layer in ((1, 2) if (rep == 0 or l2_reps) else (1,)):
                    F = 2 if layer == 1 else HID
                    w_t = w1_t if layer == 1 else w2_t
                    b_t = b1_t if layer == 1 else b2_t
                    full = rep == 0 or part == "all"

                    def tails(g, acc):
                        # per-slot tails (deferred one group for pipelining)
                        a2g = wpool.tile([P, GSLOTS * HID], BF16, tag="a2")
                        nc.scalar.mul(a2g[:], acc[:], onecol_t[:])
                        for sloc in range(GSLOTS):
                            s = g * GSLOTS + sloc
                            tp = ptp.tile([F, P], BF16, space="PSUM",
                                          tag="tp")
                            nc.tensor.transpose(
                                out=tp[:],
                                in_=a2g[:, sloc * HID:sloc * HID + F],
                                identity=ident_bf_t[:])
                            a2t = wpool.tile([F, P], BF16, tag="a2t")
                            nc.scalar.mul(a2t[:], tp[:], onecol_t[0:F, :])
                            hps = ptp.tile([P, HID], F32, space="PSUM",
                                           tag="hps")
                            nc.tensor.matmul(out=hps[:], lhsT=ones_bf_t[:],
                                             rhs=b_t[:], start=True,
                                             stop=False)
                            nc.tensor.matmul(out=hps[:], lhsT=a2t[:],
                                             rhs=w_t[:], start=False,
                                             stop=True)
                            if layer == 1:
                                nc.scalar.activation(
                                    out=h1own_t[:, HID * s:HID * (s + 1)],
                                    in_=hps[:],
                                    func=mybir.ActivationFunctionType.Relu)
                                if s < W_SLOTS:
                                    nc.sync.dma_start(
                                        out=h1slice[s * P:(s + 1) * P, 0:HID],
                                        in_=h1own_t[:, HID * s:HID * (s + 1)])
                                    if dump:
                                        nc.sync.dma_start(
                                            out=h1dump[s * P:(s + 1) * P, :],
                                            in_=h1own_t[:,
                                                        HID * s:HID * (s + 1)])
                            else:
                                h2e = wpool.tile([P, HID + 1], BF16,
                                                 tag="h2e")
                                nc.scalar.activation(
                                    out=h2e[:, 0:HID], in_=hps[:],
                                    func=mybir.ActivationFunctionType.Relu)
                                nc.vector.memset(h2e[:, HID:HID + 1], 1.0)
                                goh = ohpool.tile([P, GMAX], BF16, tag="goh")
                                nc.vector.tensor_scalar(
                                    out=goh[:], in0=iota_t[:, 0:GMAX],
                                    scalar1=g_rel_t[:, s:s + 1], scalar2=None,
                                    op0=mybir.AluOpType.is_equal)
                                nc.tensor.matmul(out=pool_ps[:], lhsT=goh[:],
                                                 rhs=h2e[:], start=(s == 0),
                                                 stop=(s == W_PAD - 1))

                    pending = None
                    for g in range(NGRP):
                        gtiles = {}
                        for (cg, ch, blk0, nb) in calls:
                            if cg != g:
                                continue
                            g_t = gpool.tile([P, nb, P], BF16, tag=f"g{ch}")
                            table = xtab if layer == 1 else h1tab
                            r0 = ch * CHUNK_ROWS
                            r1 = min(r0 + CHUNK_ROWS, NTAB)
                            if full or part == "gather":
                                nc.gpsimd.dma_gather(
                                    out_ap=g_t[:],
                                    in_ap=table[r0:r1, :],
                                    idxs_ap=idx_t[:, blk0 * 8:(blk0 + nb) * 8],
                                    num_idxs=nb * P, num_idxs_reg=nb * P,
                                    elem_size=P, single_packet=False,
                                    queue_num=ch)
                                gtiles[ch] = g_t
                            else:
                                gtiles[ch] = dumtiles[ch]
                        if not full and part == "gather":
                            continue
                        if not full and part in ("mm", "ohonly", "tails"):
                            pass

                        acc = pacc.tile([P, GSLOTS * HID], F32, space="PSUM",
                                        tag="acc")
                        mms = [m for m in mm_list if m[1] == g]
                        if not full and part == "tails":
                            mms = [m for m in mms if m[0] == "self"]
                        for mm in mms:
                            kind, _, s, blk, ci, kloc = mm[:6]
                            sloc = s - g * GSLOTS
                            stop = mm[-1] == "stop"
                            oh = ohpool.tile([P, P], BF16, tag="oh")
                            if kind == "self":
                                nc.vector.tensor_scalar(
                                    out=oh[:], in0=iota_t[:],
                                    scalar1=self_rel_t[:],
                                    scalar2=dinv2_t[:, s:s + 1],
                                    op0=mybir.AluOpType.is_equal,
                                    op1=mybir.AluOpType.mult)
                                if layer == 1:
                                    rhs = x_own_t[:, 2 * s:2 * s + 2]
                                else:
                                    rhs = h1own_t[:, HID * s:HID * (s + 1)]
                                nc.tensor.matmul(
                                    out=acc[:, sloc * HID:sloc * HID + F],
                                    lhsT=oh[:], rhs=rhs,
                                    start=True,
                                    stop=stop or (not full and
                                                  part == "tails"))
                            else:
                                col = mm[6]
                                call_ch = calls[ci][1]
                                nc.vector.tensor_scalar(
                                    out=oh[:], in0=iota_t[:],
                                    scalar1=dst_rel_t[:, col:col + 1],
                                    scalar2=dsn_t[:, blk:blk + 1],
                                    op0=mybir.AluOpType.is_equal,
                                    op1=mybir.AluOpType.mult)
                                if full or part == "mm":
                                    nc.tensor.matmul(
                                        out=acc[:, sloc * HID:sloc * HID + F],
                                        lhsT=oh[:],
                                        rhs=gtiles[call_ch][:, kloc, 0:F],
                                        start=False, stop=stop)

                        # ---- tails, deferred one group
                        if not full and part in ("mm", "ohonly"):
                            continue
                        if pending is not None:
                            tails(*pending)
                        pending = (g, acc)
                    if pending is not None:
                        tails(*pending)

                    if layer == 1 and (rep == 0 or coll_reps):
                        nc.gpsimd.collective_compute(
                            kind="AllGather",
                            op=mybir.AluOpType.bypass,
                            replica_groups=[list(range(NCORES))],
                            ins=[h1slice[:]],
                            outs=[h1tab[:]],
                        )

                # ---- mean-pool + MLP head
                pool_sb = wpool.tile([GMAX, HID + 1], F32, tag="pool")
                nc.vector.tensor_copy(out=pool_sb[:], in_=pool_ps[:])
                cntm = wpool.tile([GMAX, 1], F32, tag="cnt")
                nc.vector.tensor_scalar(
                    out=cntm[:], in0=pool_sb[:, HID:HID + 1], scalar1=1.0,
                    scalar2=None, op0=mybir.AluOpType.max)
                rcnt = wpool.tile([GMAX, 1], F32, tag="rcnt")
                nc.vector.reciprocal(out=rcnt[:], in_=cntm[:])
                means = wpool.tile([GMAX, HID], F32, tag="means")
                nc.scalar.mul(means[:], pool_sb[:, 0:HID], rcnt[:])
                mt_ps = ptp.tile([HID, GMAX], F32, space="PSUM", tag="tp")
                nc.tensor.transpose(out=mt_ps[:], in_=means[:],
                                    identity=ident_t[0:GMAX, 0:GMAX])
                mt = wpool.tile([HID, GMAX], F32, tag="mt")
                nc.vector.tensor_copy(out=mt[:], in_=mt_ps[:])
                f1_ps = ptp.tile([GMAX, HID], F32, space="PSUM", tag="hps")
                nc.tensor.matmul(out=f1_ps[:], lhsT=ones_f_t[:, 0:GMAX],
                                 rhs=bf1_t[:], start=True, stop=False)
                nc.tensor.matmul(out=f1_ps[:], lhsT=mt[:], rhs=wf1_t[:],
                                 start=False, stop=True)
                f1 = wpool.tile([GMAX, HID], F32, tag="f1")
                nc.scalar.activation(out=f1[:], in_=f1_ps[:],
                                     func=mybir.ActivationFunctionType.Relu)
                f1t_ps = ptp.tile([HID, GMAX], F32, space="PSUM", tag="tp")
                nc.tensor.transpose(out=f1t_ps[:], in_=f1[:],
                                    identity=ident_t[0:GMAX, 0:GMAX])
                f1t = wpool.tile([HID, GMAX], F32, tag="f1t")
                nc.vector.tensor_copy(out=f1t[:], in_=f1t_ps[:])
                o_ps = ptp.tile([GMAX, 4], F32, space="PSUM", tag="hps")
                nc.tensor.matmul(out=o_ps[:], lhsT=ones_f_t[:, 0:GMAX],
                                 rhs=bf2_t[:], start=True, stop=False)
                nc.tensor.matmul(out=o_ps[:], lhsT=f1t[:], rhs=wf2_t[:],
                                 start=False, stop=True)
                o_sb = wpool.tile([GMAX, 4], F32, tag="osb")
                nc.vector.tensor_copy(out=o_sb[:], in_=o_ps[:])
                nc.sync.dma_start(out=out_t[:], in_=o_sb[:])

    nc.finalize()
    return nc


# ---------------------------------------------------------------- pjrt run

class _Runner:
    def __init__(self, nc, n_cores: int = NCORES, donate: bool = True):
        import jax
        from jax.sharding import Mesh, NamedSharding, PartitionSpec
        from jax.experimental.shard_map import shard_map
        from concourse.bass2jax import (
            _bass_exec_p, install_neuronx_cc_hook, partition_id_tensor)

        install_neuronx_cc_hook()
        self.jax = jax
        self.n_cores = n_cores
        in_names, out_names, out_avals = [], [], []
        pname = nc.partition_id_tensor.name if nc.partition_id_tensor else None
        for alloc in nc.m.functions[0].allocations:
            if not isinstance(alloc, mybir.MemoryLocationSet):
                continue
            name = alloc.memorylocations[0].name
            if alloc.kind == "ExternalInput":
                if name != pname:
                    in_names.append(name)
            elif alloc.kind == "ExternalOutput":
                out_names.append(name)
                out_avals.append(jax.core.ShapedArray(
                    tuple(alloc.tensor_shape), mybir.dt.np(alloc.dtype)))
        self.in_names, self.out_names, self.out_avals = (
            in_names, out_names, out_avals)
        n_params, n_outs = len(in_names), len(out_avals)
        all_in = in_names + out_names + ([pname] if pname else [])

        def _body(*args):
            operands = list(args)
            if pname:
                operands.append(partition_id_tensor())
            return tuple(_bass_exec_p.bind(
                *operands, out_avals=tuple(out_avals),
                in_names=tuple(all_in), out_names=tuple(out_names),
                lowering_input_output_aliases=(),
                sim_require_finite=False, sim_require_nnan=False, nc=nc))

        devices = jax.devices()[:n_cores]
        self.mesh = Mesh(np.asarray(devices), ("core",))
        self.sh = NamedSharding(self.mesh, PartitionSpec("core"))
        smapped = shard_map(
            _body, mesh=self.mesh,
            in_specs=(PartitionSpec("core"),) * (n_params + n_outs),
            out_specs=(PartitionSpec("core"),) * n_outs,
            check_rep=False)
        if donate:
            self.fn = jax.jit(
                smapped,
                donate_argnums=tuple(range(n_params, n_params + n_outs)),
                keep_unused=True)
        else:
            self.fn = jax.jit(smapped, keep_unused=True)
        self.donate = donate
        self._zs = [(n_cores * a.shape[0], *a.shape[1:]) for a in out_avals]
        self._zd = [a.dtype for a in out_avals]
        self._dev_in = None
        self._zeros = None

    def stage(self, in_maps):
        ci = [np.concatenate([np.ascontiguousarray(in_maps[c][n])
                              for c in range(self.n_cores)], axis=0)
              for n in self.in_names]
        self._dev_in = [self.jax.device_put(x, self.sh) for x in ci]
        for x in self._dev_in:
            x.block_until_ready()
        if not self.donate:
            self._zeros = [self.jax.device_put(np.zeros(s, d), self.sh)
                           for s, d in zip(self._zs, self._zd)]
            for z in self._zeros:
                z.block_until_ready()

    def run(self):
        if self.donate:
            zeros = [self.jax.device_put(np.zeros(s, d), self.sh)
                     for s, d in zip(self._zs, self._zd)]
        else:
            zeros = self._zeros
        outs = self.fn(*self._dev_in, *zeros)
        for o in outs:
            o.block_until_ready()
        return outs

    def results(self, outs):
        res = []
        for c in range(self.n_cores):
            d = {}
            for i, n in enumerate(self.out_names):
                a = np.asarray(outs[i]).reshape(
                    self.n_cores, *self.out_avals[i].shape)
                d[n] = a[c]
            res.append(d)
        return res


# ----------------------------------------------------------------- kernel()

_CACHE = {}
last_run_info = {}


def _consts():
    iota = np.tile(np.arange(P, dtype=np.float32), (P, 1)).astype(NPBF)
    ident_bf = np.eye(P, dtype=np.float32).astype(NPBF)
    self_rel = np.arange(P, dtype=np.float32).reshape(P, 1)
    ident = np.eye(P, dtype=np.float32)
    ones_bf = np.ones((1, P), NPBF)
    ones_f = np.ones((1, P), np.float32)
    return iota, ident_bf, self_rel, ident, ones_bf, ones_f


def _maps(struct, percore, x, W1, b1, W2, b2, Wf1, bf1, Wf2, bf2):
    iota, ident_bf, self_rel, ident, ones_bf, ones_f = _consts()
    NTAB, W_PAD = struct["NTAB"], struct["W_PAD"]
    r2, n0s = struct["r2"], struct["n0s"]

    xtab = np.zeros((NTAB, P), NPBF)
    xtab[r2, 0:2] = x.astype(NPBF)

    maps = []
    for c in range(NCORES):
        pc = percore[c]
        n0, nown = pc["n0"], pc["nown"]
        x_own = np.zeros((P, 2 * W_PAD), NPBF)
        ar = np.arange(nown)
        x_own[ar % P, 2 * (ar >> 7)] = x[n0:n0 + nown, 0].astype(NPBF)
        x_own[ar % P, 2 * (ar >> 7) + 1] = x[n0:n0 + nown, 1].astype(NPBF)
        maps.append({
            "xtab": xtab, "idx": pc["idx_packed"], "dst_rel": pc["dst_rel"],
            "dsnp": pc["dsnp"], "deg_own": pc["deg_own"],
            "g_rel": pc["g_rel"], "x_own": x_own,
            "iota": iota, "ident_bf": ident_bf,
            "self_rel": self_rel, "ident": ident,
            "ones_bf": ones_bf, "ones_f": ones_f,
            "W1": W1.astype(NPBF), "b1": b1.astype(NPBF),
            "W2": W2.astype(NPBF), "b2": b2.astype(NPBF),
            "Wf1": Wf1, "bf1": bf1, "Wf2": Wf2, "bf2": bf2,
        })
    return maps


def kernel(x, edge_index, batch, num_graphs=None, W1=None, b1=None, W2=None,
           b2=None, Wf1=None, bf1=None, Wf2=None, bf2=None):
    x = np.asarray(x, dtype=np.float32)
    W1 = np.asarray(W1, dtype=np.float32)
    b1 = np.asarray(b1, dtype=np.float32).reshape(1, HID)
    W2 = np.asarray(W2, dtype=np.float32)
    b2 = np.asarray(b2, dtype=np.float32).reshape(1, HID)
    Wf1 = np.asarray(Wf1, dtype=np.float32)
    bf1 = np.asarray(bf1, dtype=np.float32).reshape(1, HID)
    Wf2 = np.asarray(Wf2, dtype=np.float32)
    bf2 = np.asarray(bf2, dtype=np.float32).reshape(1, 4)

    ei = np.asarray(edge_index)
    bt = np.asarray(batch)
    key = hash((ei.tobytes(), bt.tobytes()))
    if key not in _CACHE:
        t0 = time.time()
        struct, percore = _prep(ei, bt)
        nc1 = _build(struct, reps=1)
        r1 = _Runner(nc1)
        _CACHE[key] = (struct, percore, r1)
        last_run_info["build_s"] = time.time() - t0
    struct, percore, r1 = _CACHE[key]

    maps = _maps(struct, percore, x, W1, b1, W2, b2, Wf1, bf1, Wf2, bf2)
    t0 = time.time()
    r1.stage(maps)
    last_run_info["stage_s"] = time.time() - t0
    t0 = time.time()
    outs = r1.run()
    last_run_info["run_s"] = time.time() - t0
    res = r1.results(outs)

    out = np.zeros((N_GRAPHS, 4), dtype=np.float32)
    for c in range(NCORES):
        pc = percore[c]
        out[pc["g0"]:pc["g1"]] = res[c]["out"][0:pc["g1"] - pc["g0"]]

    last_run_info["maps"] = maps
    last_run_info["key"] = key
    return out


def measure_hw_ns(burst: int = 16):
    """On-device exec time via work-multiplying slope: amortized wall of a
    reps=R program minus the reps=1 program, divided by (R-1). Cancels all
    dispatch and transfer overheads; measures only the on-device time of one
    full pass (both GCN layers, AllGather exchange, pool + MLP head).
    Bursts of the two programs are interleaved and the median paired slope
    is reported to suppress drift on the shared device."""
    import time as _t
    key = last_run_info["key"]
    struct, percore, r1 = _CACHE[key]
    maps = last_run_info["maps"]
    R = 5

    runners = {}
    for reps in (1, R):
        ck = ("timing", reps)
        if ck not in _CACHE:
            nct = _build(struct, reps=reps)
            rx = _Runner(nct, donate=False)
            rx.stage(maps)
            _CACHE[ck] = rx
        runners[reps] = _CACHE[ck]
        runners[reps].run()  # warm

    def _burst(rx):
        t0 = _t.perf_counter()
        outs = None
        for _ in range(burst):
            outs = rx.fn(*rx._dev_in, *rx._zeros)
        for o in outs:
            o.block_until_ready()
        return (_t.perf_counter() - t0) / burst

    slopes = []
    detail = []
    for _ in range(5):
        t1 = _burst(runners[1])
        tR = _burst(runners[R])
        t1b = _burst(runners[1])
        slopes.append((tR - min(t1, t1b)) / (R - 1))
        detail.append((round(t1 * 1e6), round(tR * 1e6), round(t1b * 1e6)))
    slopes.sort()
    hw_s = slopes[len(slopes) // 2]
    last_run_info["hw_detail"] = {
        "pairs_us": detail,
        "slopes_us": [round(s * 1e6, 1) for s in slopes]}
    return max(hw_s, 0.0) * 1e9


# revision 8
# speedup vs baseline: 1.1008x; 1.1008x over previous
"""GCN graph classifier on 8 Trainium2 NeuronCores (Bass/Tile) — single launch.

v2 design (graph/data parallel per the sharding hint):
- Nodes split into 8 contiguous graph-aligned ranges; each core owns the
  destination side of every edge landing in its range, the pooling and the
  MLP head for its graphs.
- Both GCN layers, the inter-layer hidden-state exchange (DRAM AllGather),
  the mean-pool and the MLP head run in ONE device launch per kernel call.
- Tables live in DRAM in "slice layout" (row r2(n) = owner*W_SLOTS*128 +
  (n - n0[owner])), 256-byte bf16 rows; per-edge rows are fetched with
  dma_gather (int16 indices, 4 chunk windows of 32768 rows).
- Scatter-accumulate uses one-hot matmuls: per 128-edge block, a bf16
  one-hot built on DVE (iota == dst_rel, scaled by rsqrt(deg_src*deg_dst))
  feeds a PE matmul accumulating into a per-slot PSUM window. Blocks are
  packed without per-slot padding; a block whose edges straddle slot
  boundaries issues one matmul per touched slot.
- Self-loop terms use resident SBUF tiles (x / layer-1 hidden) with a
  diagonal one-hot of 1/deg, skipping ~25k gather descriptors per core.
- All per-core variation (indices, one-hot selectors, degrees) is input
  data; the compiled program is identical across cores (SPMD).

Self-contained: no imports from the problem directory.
"""
import time

import numpy as np

import concourse.bass as bass
import concourse.bacc as bacc
import concourse.mybir as mybir
import concourse.tile as tile

N_NODES = 100_000
N_EDGES = 1_200_000
N_GRAPHS = 512
HID = 64
NCORES = 8
P = 128
CHUNK_ROWS = 32_768
GSLOTS = 8                # slots per PSUM group
GMAX = 96                 # padded graphs per core
SENT = 30_000.0           # one-hot sentinel (bf16 -> 29952, never matches)

F32 = mybir.dt.float32
BF16 = mybir.dt.bfloat16
I16 = mybir.dt.int16
NPBF = mybir.dt.np(BF16)


# ---------------------------------------------------------------- host prep

def _prep(edge_index: np.ndarray, batch: np.ndarray):
    """Index-side preprocessing only: integer index manipulation derived from
    the graph structure plus integer degree counts (rsqrt happens on device)."""
    src = np.asarray(edge_index[0], dtype=np.int64)
    dst = np.asarray(edge_index[1], dtype=np.int64)
    batch = np.asarray(batch, dtype=np.int64)

    deg = np.bincount(dst, minlength=N_NODES) + 1  # +1 self-loop

    gptr = np.searchsorted(batch, np.arange(N_GRAPHS + 1))
    targets = (np.arange(1, NCORES) * N_NODES) // NCORES
    gsplit = np.searchsorted(gptr, targets)
    g0s = np.concatenate([[0], gsplit, [N_GRAPHS]])
    n0s = gptr[g0s]  # len 9
    nowns = np.diff(n0s)
    W_SLOTS = int(max(-(-n // P) for n in nowns))
    NTAB = NCORES * W_SLOTS * P
    NCHUNK = -(-NTAB // CHUNK_ROWS)
    NGRP = -(-W_SLOTS // GSLOTS)
    W_PAD = NGRP * GSLOTS
    assert max(g0s[c + 1] - g0s[c] for c in range(NCORES)) <= GMAX

    owner = np.searchsorted(n0s[1:], np.arange(N_NODES), side="right")
    r2 = owner * (W_SLOTS * P) + (np.arange(N_NODES) - n0s[owner])

    NCELL = NGRP * NCHUNK
    cores_raw = []
    cnt = np.zeros((NCORES, NCELL), dtype=np.int64)
    for c in range(NCORES):
        n0, n1 = int(n0s[c]), int(n0s[c + 1])
        m = (dst >= n0) & (dst < n1)
        es, ed = src[m], dst[m]
        slot = (ed - n0) >> 7
        grp = slot >> 3
        chunk = r2[es] >> 15
        o = np.lexsort((ed, slot, chunk, grp))
        es, ed, slot, grp, chunk = es[o], ed[o], slot[o], grp[o], chunk[o]
        cell = grp * NCHUNK + chunk
        cnt[c] = np.bincount(cell, minlength=NCELL)
        cores_raw.append((n0, n1, es, ed, slot, cell))

    K = -(-cnt.max(axis=0) // P)          # blocks per cell (0 stays 0)
    block_base = np.concatenate([[0], np.cumsum(K)[:-1]])
    NBLK = int(K.sum())
    NE = NBLK * P

    # calls: one gather per non-empty (group, chunk)
    calls = []  # (g, ch, blk0, nblk)
    for g in range(NGRP):
        for ch in range(NCHUNK):
            cell = g * NCHUNK + ch
            if K[cell] > 0:
                calls.append((g, ch, int(block_base[cell]), int(K[cell])))
    call_of_cell = {}
    for ci, (g, ch, b0, nb) in enumerate(calls):
        call_of_cell[g * NCHUNK + ch] = ci

    # per-core edge placement + per-block slot spans
    lo = np.full(NBLK, 1 << 30, dtype=np.int64)
    hi = np.full(NBLK, -1, dtype=np.int64)
    percore_edges = []
    for c in range(NCORES):
        n0, n1, es, ed, slot, cell = cores_raw[c]
        cell_start = np.concatenate([[0], np.cumsum(np.bincount(
            cell, minlength=NCELL))])[:-1]
        rank = np.arange(len(es)) - cell_start[cell]
        blk = block_base[cell] + (rank >> 7)
        pos = blk * P + (rank & 127)
        np.minimum.at(lo, blk, slot)
        np.maximum.at(hi, blk, slot)
        percore_edges.append((n0, es, ed, slot, blk, rank & 127))
    assert (hi >= 0).all()

    # mm schedule (shared across cores). PSUM accumulation groups to the same
    # bank must be contiguous in PE issue order, so emit slot-major: each
    # slot's self mm + edge mms form one contiguous start..stop group.
    mm_list = []   # (kind, g, s, blk, ci, kloc, col) col=dst_rel column
    NMME = 0
    for g in range(NGRP):
        per_slot = {s: [] for s in range(g * GSLOTS, (g + 1) * GSLOTS)}
        for ch in range(NCHUNK):
            cell = g * NCHUNK + ch
            for k in range(int(K[cell])):
                blk = int(block_base[cell]) + k
                for s in range(int(lo[blk]), int(hi[blk]) + 1):
                    per_slot[s].append((blk, call_of_cell[cell], k))
        for s in range(g * GSLOTS, (g + 1) * GSLOTS):
            mm_list.append(["self", g, s, -1, -1, -1, -1])
            for blk, ci, k in per_slot[s]:
                mm_list.append(["edge", g, s, blk, ci, k, NMME])
                NMME += 1
            mm_list[-1].append("stop")
    NMM = len(mm_list)

    # per-core data arrays
    rng_pad = np.random.default_rng(12345)
    percore = []
    for c in range(NCORES):
        n0, es, ed, slot, blk, wpos = percore_edges[c]

        idx_flat = rng_pad.integers(0, 2048, size=NE).astype(np.int16)
        idx_flat[blk * P + wpos] = (r2[es] & (CHUNK_ROWS - 1)).astype(np.int16)
        cols = NE // 16
        arr = np.zeros((16, cols), dtype=np.int16)
        j = np.arange(NE)
        arr[j % 16, j // 16] = idx_flat
        idx_packed = np.tile(arr, (8, 1))

        # dst_rel: one column per edge mm
        mme_of_blk_s = {}
        for mi, mm in enumerate(mm_list):
            if mm[0] == "edge":
                mme_of_blk_s[(mm[3], mm[2])] = mm[6]
        col = np.array([mme_of_blk_s[(b, s)] for b, s in zip(blk, slot)])
        dst_rel = np.full((P, NMME), SENT, dtype=np.float32)
        dst_rel[wpos, col] = (ed - n0 - slot * P).astype(np.float32)

        dsnp = np.ones((P, NBLK), dtype=np.float32)
        dsnp[wpos, blk] = (deg[es] * deg[ed]).astype(np.float32)

        nown = int(n0s[c + 1]) - n0
        ar = np.arange(nown)
        deg_own = np.ones((P, W_PAD), dtype=np.float32)
        deg_own[ar % P, ar >> 7] = deg[n0:n0 + nown].astype(np.float32)
        g_rel = np.full((P, W_PAD), SENT, dtype=np.float32)
        g_rel[ar % P, ar >> 7] = (batch[n0:n0 + nown] - g0s[c]).astype(
            np.float32)

        percore.append({
            "n0": n0, "nown": nown, "g0": int(g0s[c]), "g1": int(g0s[c + 1]),
            "idx_packed": idx_packed,
            "dst_rel": dst_rel,
            "dsnp": dsnp,
            "deg_own": deg_own,
            "g_rel": g_rel,
        })

    struct = {
        "W_SLOTS": W_SLOTS, "W_PAD": W_PAD, "NTAB": NTAB, "NGRP": NGRP,
        "NCHUNK": NCHUNK, "NBLK": NBLK, "NE": NE, "NMM": NMM, "NMME": NMME,
        "calls": calls, "mm_list": mm_list, "r2": r2, "n0s": n0s, "g0s": g0s,
    }
    return struct, percore


# ------------------------------------------------------------- bass program

def _build(struct, reps: int = 1, dump: bool = False,
           coll_reps: bool = True, l2_reps: bool = True, part: str = 'all'):
    W_SLOTS, W_PAD = struct["W_SLOTS"], struct["W_PAD"]
    NTAB, NGRP, NCHUNK = struct["NTAB"], struct["NGRP"], struct["NCHUNK"]
    NBLK, NE, NMME = struct["NBLK"], struct["NE"], struct["NMME"]
    calls, mm_list = struct["calls"], struct["mm_list"]

    nc = bacc.Bacc("TRN2", num_swdge_queues=4)
    xtab = nc.dram_tensor("xtab", (NTAB, P), BF16, kind="ExternalInput")
    idx_in = nc.dram_tensor("idx", (P, NE // 16), I16, kind="ExternalInput")
    dst_rel_in = nc.dram_tensor("dst_rel", (P, NMME), F32,
                                kind="ExternalInput")
    dsnp_in = nc.dram_tensor("dsnp", (P, NBLK), F32, kind="ExternalInput")
    deg_own_in = nc.dram_tensor("deg_own", (P, W_PAD), F32,
                                kind="ExternalInput")
    g_rel_in = nc.dram_tensor("g_rel", (P, W_PAD), F32, kind="ExternalInput")
    x_own_in = nc.dram_tensor("x_own", (P, 2 * W_PAD), BF16,
                              kind="ExternalInput")
    iota_in = nc.dram_tensor("iota", (P, P), BF16, kind="ExternalInput")
    self_rel_in = nc.dram_tensor("self_rel", (P, 1), F32,
                                 kind="ExternalInput")
    ident_in = nc.dram_tensor("ident", (P, P), F32, kind="ExternalInput")
    ident_bf_in = nc.dram_tensor("ident_bf", (P, P), BF16,
                                 kind="ExternalInput")
    ones_bf_in = nc.dram_tensor("ones_bf", (1, P), BF16, kind="ExternalInput")
    ones_f_in = nc.dram_tensor("ones_f", (1, P), F32, kind="ExternalInput")
    w1_in = nc.dram_tensor("W1", (2, HID), BF16, kind="ExternalInput")
    b1_in = nc.dram_tensor("b1", (1, HID), BF16, kind="ExternalInput")
    w2_in = nc.dram_tensor("W2", (HID, HID), BF16, kind="ExternalInput")
    b2_in = nc.dram_tensor("b2", (1, HID), BF16, kind="ExternalInput")
    wf1_in = nc.dram_tensor("Wf1", (HID, HID), F32, kind="ExternalInput")
    bf1_in = nc.dram_tensor("bf1", (1, HID), F32, kind="ExternalInput")
    wf2_in = nc.dram_tensor("Wf2", (HID, 4), F32, kind="ExternalInput")
    bf2_in = nc.dram_tensor("bf2", (1, 4), F32, kind="ExternalInput")

    h1slice = nc.dram_tensor("h1slice", (W_SLOTS * P, P), BF16)
    if dump:
        h1dump = nc.dram_tensor("h1dump", (W_SLOTS * P, HID), BF16,
                                kind="ExternalOutput")
    h1tab = nc.dram_tensor("h1tab", (NTAB, P), BF16)
    out_t = nc.dram_tensor("out", (GMAX, 4), F32, kind="ExternalOutput")

    with tile.TileContext(nc) as tc:
        with tc.tile_pool(name="const", bufs=1) as cpool, \
             tc.tile_pool(name="meta", bufs=1) as mpool, \
             tc.tile_pool(name="gat", bufs=3) as gpool, \
             tc.tile_pool(name="work", bufs=3) as wpool, \
             tc.tile_pool(name="oh", bufs=48) as ohpool, \
             tc.tile_pool(name="pacc", bufs=3, space="PSUM") as pacc, \
             tc.tile_pool(name="ptp", bufs=2, space="PSUM") as ptp, \
             tc.tile_pool(name="ppool", bufs=1, space="PSUM") as ppool:

            # ---- constants / metadata
            iota_t = cpool.tile([P, P], BF16)
            nc.sync.dma_start(out=iota_t[:], in_=iota_in[:])
            self_rel_t = cpool.tile([P, 1], F32)
            nc.sync.dma_start(out=self_rel_t[:], in_=self_rel_in[:])
            ident_t = cpool.tile([P, P], F32)
            nc.sync.dma_start(out=ident_t[:], in_=ident_in[:])
            ident_bf_t = cpool.tile([P, P], BF16)
            nc.sync.dma_start(out=ident_bf_t[:], in_=ident_bf_in[:])
            ones_bf_t = cpool.tile([1, P], BF16)
            nc.sync.dma_start(out=ones_bf_t[:], in_=ones_bf_in[:])
            ones_f_t = cpool.tile([1, P], F32)
            nc.sync.dma_start(out=ones_f_t[:], in_=ones_f_in[:])
            onecol_t = cpool.tile([P, 1], F32)
            nc.vector.memset(onecol_t[:], 1.0)
            w1_t = cpool.tile([2, HID], BF16)
            nc.sync.dma_start(out=w1_t[:], in_=w1_in[:])
            b1_t = cpool.tile([1, HID], BF16)
            nc.sync.dma_start(out=b1_t[:], in_=b1_in[:])
            w2_t = cpool.tile([HID, HID], BF16)
            nc.sync.dma_start(out=w2_t[:], in_=w2_in[:])
            b2_t = cpool.tile([1, HID], BF16)
            nc.sync.dma_start(out=b2_t[:], in_=b2_in[:])
            wf1_t = cpool.tile([HID, HID], F32)
            nc.sync.dma_start(out=wf1_t[:], in_=wf1_in[:])
            bf1_t = cpool.tile([1, HID], F32)
            nc.sync.dma_start(out=bf1_t[:], in_=bf1_in[:])
            wf2_t = cpool.tile([HID, 4], F32)
            nc.sync.dma_start(out=wf2_t[:], in_=wf2_in[:])
            bf2_t = cpool.tile([1, 4], F32)
            nc.sync.dma_start(out=bf2_t[:], in_=bf2_in[:])

            idx_t = mpool.tile([P, NE // 16], I16)
            nc.sync.dma_start(out=idx_t[:], in_=idx_in[:])
            dst_rel_t = mpool.tile([P, NMME], F32)
            nc.sync.dma_start(out=dst_rel_t[:], in_=dst_rel_in[:])
            g_rel_t = mpool.tile([P, W_PAD], F32)
            nc.sync.dma_start(out=g_rel_t[:], in_=g_rel_in[:])
            x_own_t = mpool.tile([P, 2 * W_PAD], BF16)
            nc.sync.dma_start(out=x_own_t[:], in_=x_own_in[:])

            # dsn = rsqrt(deg_src*deg_dst) in bf16
            dsnp_t = mpool.tile([P, NBLK], F32)
            nc.sync.dma_start(out=dsnp_t[:], in_=dsnp_in[:])
            dsnr_t = mpool.tile([P, NBLK], F32)
            nc.vector.reciprocal(out=dsnr_t[:], in_=dsnp_t[:])
            dsn_t = mpool.tile([P, NBLK], F32)
            nc.scalar.sqrt(out=dsn_t[:], in_=dsnr_t[:])

            # dinv2_own = 1/deg_own in bf16
            deg_own_t = mpool.tile([P, W_PAD], F32)
            nc.sync.dma_start(out=deg_own_t[:], in_=deg_own_in[:])
            dinv2_t = mpool.tile([P, W_PAD], F32)
            nc.vector.reciprocal(out=dinv2_t[:], in_=deg_own_t[:])

            # resident layer-1 hidden (own nodes), persists across layers
            h1own_t = mpool.tile([P, W_PAD * HID], BF16)

            dumtiles = None
            if part in ("mm", "ohonly", "tails"):
                maxnb = max(nb for (_, _, _, nb) in calls)
                dumtiles = {}
                for ch in range(NCHUNK):
                    dt_ = mpool.tile([P, maxnb, P], BF16, tag=f"dum{ch}")
                    nc.vector.memset(dt_[:], 0.125)
                    dumtiles[ch] = dt_

            pool_ps = ppool.tile([GMAX, HID + 1], F32, space="PSUM")

            for rep in range(reps):
                for layer in ((1, 2) if (rep == 0 or l2_reps) else (1,)):
                    F = 2 if layer == 1 else HID
                    w_t = w1_t if layer == 1 else w2_t
                    b_t = b1_t if layer == 1 else b2_t
                    full = rep == 0 or part == "all"

                    def tails(g, acc):
                        # per-slot tails (deferred one group for pipelining)
                        a2g = wpool.tile([P, GSLOTS * HID], BF16, tag="a2")
                        nc.scalar.mul(a2g[:], acc[:], onecol_t[:])
                        for sloc in range(GSLOTS):
                            s = g * GSLOTS + sloc
                            tp = ptp.tile([F, P], BF16, space="PSUM",
                                          tag="tp")
                            nc.tensor.transpose(
                                out=tp[:],
                                in_=a2g[:, sloc * HID:sloc * HID + F],
                                identity=ident_bf_t[:])
                            a2t = wpool.tile([F, P], BF16, tag="a2t")
                            nc.scalar.mul(a2t[:], tp[:], onecol_t[0:F, :])
                            hps = ptp.tile([P, HID], F32, space="PSUM",
                                           tag="hps")
                            nc.tensor.matmul(out=hps[:], lhsT=ones_bf_t[:],
                                             rhs=b_t[:], start=True,
                                             stop=False)
                            nc.tensor.matmul(out=hps[:], lhsT=a2t[:],
                                             rhs=w_t[:], start=False,
                                             stop=True)
                            if layer == 1:
                                nc.scalar.activation(
                                    out=h1own_t[:, HID * s:HID * (s + 1)],
                                    in_=hps[:],
                                    func=mybir.ActivationFunctionType.Relu)
                                if s < W_SLOTS:
                                    nc.sync.dma_start(
                                        out=h1slice[s * P:(s + 1) * P, 0:HID],
                                        in_=h1own_t[:, HID * s:HID * (s + 1)])
                                    if dump:
                                        nc.sync.dma_start(
                                            out=h1dump[s * P:(s + 1) * P, :],
                                            in_=h1own_t[:,
                                                        HID * s:HID * (s + 1)])
                            else:
                                h2e = wpool.tile([P, HID + 1], BF16,
                                                 tag="h2e")
                                nc.scalar.activation(
                                    out=h2e[:, 0:HID], in_=hps[:],
                                    func=mybir.ActivationFunctionType.Relu)
                                nc.vector.memset(h2e[:, HID:HID + 1], 1.0)
                                goh = ohpool.tile([P, GMAX], BF16, tag="goh")
                                nc.vector.tensor_scalar(
                                    out=goh[:], in0=iota_t[:, 0:GMAX],
                                    scalar1=g_rel_t[:, s:s + 1], scalar2=None,
                                    op0=mybir.AluOpType.is_equal)
                                nc.tensor.matmul(out=pool_ps[:], lhsT=goh[:],
                                                 rhs=h2e[:], start=(s == 0),
                                                 stop=(s == W_PAD - 1))

                    pending = None
                    for g in range(NGRP):
                        gtiles = {}
                        for (cg, ch, blk0, nb) in calls:
                            if cg != g:
                                continue
                            g_t = gpool.tile([P, nb, P], BF16, tag=f"g{ch}")
                            table = xtab if layer == 1 else h1tab
                            r0 = ch * CHUNK_ROWS
                            r1 = min(r0 + CHUNK_ROWS, NTAB)
                            if full or part == "gather":
                                nc.gpsimd.dma_gather(
                                    out_ap=g_t[:],
                                    in_ap=table[r0:r1, :],
                                    idxs_ap=idx_t[:, blk0 * 8:(blk0 + nb) * 8],
                                    num_idxs=nb * P, num_idxs_reg=nb * P,
                                    elem_size=P, single_packet=False,
                                    queue_num=ch)
                                gtiles[ch] = g_t
                            else:
                                gtiles[ch] = dumtiles[ch]
                        if not full and part == "gather":
                            continue
                        if not full and part in ("mm", "ohonly", "tails"):
                            pass

                        acc = pacc.tile([P, GSLOTS * HID], F32, space="PSUM",
                                        tag="acc")
                        mms = [m for m in mm_list if m[1] == g]
                        if not full and part == "tails":
                            mms = [m for m in mms if m[0] == "self"]
                        for mm in mms:
                            kind, _, s, blk, ci, kloc = mm[:6]
                            sloc = s - g * GSLOTS
                            stop = mm[-1] == "stop"
                            oh = ohpool.tile([P, P], BF16, tag="oh")
                            if kind == "self":
                                nc.vector.tensor_scalar(
                                    out=oh[:], in0=iota_t[:],
                                    scalar1=self_rel_t[:],
                                    scalar2=dinv2_t[:, s:s + 1],
                                    op0=mybir.AluOpType.is_equal,
                                    op1=mybir.AluOpType.mult)
                                if layer == 1:
                                    rhs = x_own_t[:, 2 * s:2 * s + 2]
                                else:
                                    rhs = h1own_t[:, HID * s:HID * (s + 1)]
                                nc.tensor.matmul(
                                    out=acc[:, sloc * HID:sloc * HID + F],
                                    lhsT=oh[:], rhs=rhs,
                                    start=True,
                                    stop=stop or (not full and
                                                  part == "tails"))
                            else:
                                col = mm[6]
                                call_ch = calls[ci][1]
                                nc.vector.tensor_scalar(
                                    out=oh[:], in0=iota_t[:],
                                    scalar1=dst_rel_t[:, col:col + 1],
                                    scalar2=dsn_t[:, blk:blk + 1],
                                    op0=mybir.AluOpType.is_equal,
                                    op1=mybir.AluOpType.mult)
                                if full or part == "mm":
                                    nc.tensor.matmul(
                                        out=acc[:, sloc * HID:sloc * HID + F],
                                        lhsT=oh[:],
                                        rhs=gtiles[call_ch][:, kloc, 0:F],
                                        start=False, stop=stop)

                        # ---- tails, deferred one group
                        if not full and part in ("mm", "ohonly"):
                            continue
                        if pending is not None:
                            tails(*pending)
                        pending = (g, acc)
                    if pending is not None:
                        tails(*pending)

                    if layer == 1 and (rep == 0 or coll_reps):
                        nc.gpsimd.collective_compute(
                            kind="AllGather",
                            op=mybir.AluOpType.bypass,
                            replica_groups=[list(range(NCORES))],
                            ins=[h1slice[:]],
                            outs=[h1tab[:]],
                        )

                # ---- mean-pool + MLP head
                pool_sb = wpool.tile([GMAX, HID + 1], F32, tag="pool")
                nc.vector.tensor_copy(out=pool_sb[:], in_=pool_ps[:])
                cntm = wpool.tile([GMAX, 1], F32, tag="cnt")
                nc.vector.tensor_scalar(
                    out=cntm[:], in0=pool_sb[:, HID:HID + 1], scalar1=1.0,
                    scalar2=None, op0=mybir.AluOpType.max)
                rcnt = wpool.tile([GMAX, 1], F32, tag="rcnt")
                nc.vector.reciprocal(out=rcnt[:], in_=cntm[:])
                means = wpool.tile([GMAX, HID], F32, tag="means")
                nc.scalar.mul(means[:], pool_sb[:, 0:HID], rcnt[:])
                mt_ps = ptp.tile([HID, GMAX], F32, space="PSUM", tag="tp")
                nc.tensor.transpose(out=mt_ps[:], in_=means[:],
                                    identity=ident_t[0:GMAX, 0:GMAX])
                mt = wpool.tile([HID, GMAX], F32, tag="mt")
                nc.vector.tensor_copy(out=mt[:], in_=mt_ps[:])
                f1_ps = ptp.tile([GMAX, HID], F32, space="PSUM", tag="hps")
                nc.tensor.matmul(out=f1_ps[:], lhsT=ones_f_t[:, 0:GMAX],
                                 rhs=bf1_t[:], start=True, stop=False)
                nc.tensor.matmul(out=f1_ps[:], lhsT=mt[:], rhs=wf1_t[:],
                                 start=False, stop=True)
                f1 = wpool.tile([GMAX, HID], F32, tag="f1")
                nc.scalar.activation(out=f1[:], in_=f1_ps[:],
                                     func=mybir.ActivationFunctionType.Relu)
                f1t_ps = ptp.tile([HID, GMAX], F32, space="PSUM", tag="tp")
                nc.tensor.transpose(out=f1t_ps[:], in_=f1[:],
                                    identity=ident_t[0:GMAX, 0:GMAX])
                f1t = wpool.tile([HID, GMAX], F32, tag="f1t")
                nc.vector.tensor_copy(out=f1t[:], in_=f1t_ps[:])
                o_ps = ptp.tile([GMAX, 4], F32, space="PSUM", tag="hps")
                nc.tensor.matmul(out=o_ps[:], lhsT=ones_f_t[:, 0:GMAX],
                                 rhs=bf2_t[:], start=True, stop=False)
                nc.tensor.matmul(out=o_ps[:], lhsT=f1t[:], rhs=wf2_t[:],
                                 start=False, stop=True)
                o_sb = wpool.tile([GMAX, 4], F32, tag="osb")
                nc.vector.tensor_copy(out=o_sb[:], in_=o_ps[:])
                nc.sync.dma_start(out=out_t[:], in_=o_sb[:])

    nc.finalize()
    return nc


# ---------------------------------------------------------------- pjrt run

class _Runner:
    def __init__(self, nc, n_cores: int = NCORES, donate: bool = True):
        import jax
        from jax.sharding import Mesh, NamedSharding, PartitionSpec
        from jax.experimental.shard_map import shard_map
        from concourse.bass2jax import (
            _bass_exec_p, install_neuronx_cc_hook, partition_id_tensor)

        install_neuronx_cc_hook()
        self.jax = jax
        self.n_cores = n_cores
        in_names, out_names, out_avals = [], [], []
        pname = nc.partition_id_tensor.name if nc.partition_id_tensor else None
        for alloc in nc.m.functions[0].allocations:
            if not isinstance(alloc, mybir.MemoryLocationSet):
                continue
            name = alloc.memorylocations[0].name
            if alloc.kind == "ExternalInput":
                if name != pname:
                    in_names.append(name)
            elif alloc.kind == "ExternalOutput":
                out_names.append(name)
                out_avals.append(jax.core.ShapedArray(
                    tuple(alloc.tensor_shape), mybir.dt.np(alloc.dtype)))
        self.in_names, self.out_names, self.out_avals = (
            in_names, out_names, out_avals)
        n_params, n_outs = len(in_names), len(out_avals)
        all_in = in_names + out_names + ([pname] if pname else [])

        def _body(*args):
            operands = list(args)
            if pname:
                operands.append(partition_id_tensor())
            return tuple(_bass_exec_p.bind(
                *operands, out_avals=tuple(out_avals),
                in_names=tuple(all_in), out_names=tuple(out_names),
                lowering_input_output_aliases=(),
                sim_require_finite=False, sim_require_nnan=False, nc=nc))

        devices = jax.devices()[:n_cores]
        self.mesh = Mesh(np.asarray(devices), ("core",))
        self.sh = NamedSharding(self.mesh, PartitionSpec("core"))
        smapped = shard_map(
            _body, mesh=self.mesh,
            in_specs=(PartitionSpec("core"),) * (n_params + n_outs),
            out_specs=(PartitionSpec("core"),) * n_outs,
            check_rep=False)
        if donate:
            self.fn = jax.jit(
                smapped,
                donate_argnums=tuple(range(n_params, n_params + n_outs)),
                keep_unused=True)
        else:
            self.fn = jax.jit(smapped, keep_unused=True)
        self.donate = donate
        self._zs = [(n_cores * a.shape[0], *a.shape[1:]) for a in out_avals]
        self._zd = [a.dtype for a in out_avals]
        self._dev_in = None
        self._zeros = None

    def stage(self, in_maps):
        ci = [np.concatenate([np.ascontiguousarray(in_maps[c][n])
                              for c in range(self.n_cores)], axis=0)
              for n in self.in_names]
        self._dev_in = [self.jax.device_put(x, self.sh) for x in ci]
        for x in self._dev_in:
            x.block_until_ready()
        if not self.donate:
            self._zeros = [self.jax.device_put(np.zeros(s, d), self.sh)
                           for s, d in zip(self._zs, self._zd)]
            for z in self._zeros:
                z.block_until_ready()

    def run(self):
        if self.donate:
            zeros = [self.jax.device_put(np.zeros(s, d), self.sh)
                     for s, d in zip(self._zs, self._zd)]
        else:
            zeros = self._zeros
        outs = self.fn(*self._dev_in, *zeros)
        for o in outs:
            o.block_until_ready()
        return outs

    def results(self, outs):
        res = []
        for c in range(self.n_cores):
            d = {}
            for i, n in enumerate(self.out_names):
                a = np.asarray(outs[i]).reshape(
                    self.n_cores, *self.out_avals[i].shape)
                d[n] = a[c]
            res.append(d)
        return res


# ----------------------------------------------------------------- kernel()

_CACHE = {}
last_run_info = {}


def _consts():
    iota = np.tile(np.arange(P, dtype=np.float32), (P, 1)).astype(NPBF)
    ident_bf = np.eye(P, dtype=np.float32).astype(NPBF)
    self_rel = np.arange(P, dtype=np.float32).reshape(P, 1)
    ident = np.eye(P, dtype=np.float32)
    ones_bf = np.ones((1, P), NPBF)
    ones_f = np.ones((1, P), np.float32)
    return iota, ident_bf, self_rel, ident, ones_bf, ones_f


def _maps(struct, percore, x, W1, b1, W2, b2, Wf1, bf1, Wf2, bf2):
    iota, ident_bf, self_rel, ident, ones_bf, ones_f = _consts()
    NTAB, W_PAD = struct["NTAB"], struct["W_PAD"]
    r2, n0s = struct["r2"], struct["n0s"]

    xtab = np.zeros((NTAB, P), NPBF)
    xtab[r2, 0:2] = x.astype(NPBF)

    maps = []
    for c in range(NCORES):
        pc = percore[c]
        n0, nown = pc["n0"], pc["nown"]
        x_own = np.zeros((P, 2 * W_PAD), NPBF)
        ar = np.arange(nown)
        x_own[ar % P, 2 * (ar >> 7)] = x[n0:n0 + nown, 0].astype(NPBF)
        x_own[ar % P, 2 * (ar >> 7) + 1] = x[n0:n0 + nown, 1].astype(NPBF)
        maps.append({
            "xtab": xtab, "idx": pc["idx_packed"], "dst_rel": pc["dst_rel"],
            "dsnp": pc["dsnp"], "deg_own": pc["deg_own"],
            "g_rel": pc["g_rel"], "x_own": x_own,
            "iota": iota, "ident_bf": ident_bf,
            "self_rel": self_rel, "ident": ident,
            "ones_bf": ones_bf, "ones_f": ones_f,
            "W1": W1.astype(NPBF), "b1": b1.astype(NPBF),
            "W2": W2.astype(NPBF), "b2": b2.astype(NPBF),
            "Wf1": Wf1, "bf1": bf1, "Wf2": Wf2, "bf2": bf2,
        })
    return maps


def kernel(x, edge_index, batch, num_graphs=None, W1=None, b1=None, W2=None,
           b2=None, Wf1=None, bf1=None, Wf2=None, bf2=None):
    x = np.asarray(x, dtype=np.float32)
    W1 = np.asarray(W1, dtype=np.float32)
    b1 = np.asarray(b1, dtype=np.float32).reshape(1, HID)
    W2 = np.asarray(W2, dtype=np.float32)
    b2 = np.asarray(b2, dtype=np.float32).reshape(1, HID)
    Wf1 = np.asarray(Wf1, dtype=np.float32)
    bf1 = np.asarray(bf1, dtype=np.float32).reshape(1, HID)
    Wf2 = np.asarray(Wf2, dtype=np.float32)
    bf2 = np.asarray(bf2, dtype=np.float32).reshape(1, 4)

    ei = np.asarray(edge_index)
    bt = np.asarray(batch)
    key = hash((ei.tobytes(), bt.tobytes()))
    if key not in _CACHE:
        t0 = time.time()
        struct, percore = _prep(ei, bt)
        nc1 = _build(struct, reps=1)
        r1 = _Runner(nc1)
        _CACHE[key] = (struct, percore, r1)
        last_run_info["build_s"] = time.time() - t0
    struct, percore, r1 = _CACHE[key]

    maps = _maps(struct, percore, x, W1, b1, W2, b2, Wf1, bf1, Wf2, bf2)
    t0 = time.time()
    r1.stage(maps)
    last_run_info["stage_s"] = time.time() - t0
    t0 = time.time()
    outs = r1.run()
    last_run_info["run_s"] = time.time() - t0
    res = r1.results(outs)

    out = np.zeros((N_GRAPHS, 4), dtype=np.float32)
    for c in range(NCORES):
        pc = percore[c]
        out[pc["g0"]:pc["g1"]] = res[c]["out"][0:pc["g1"] - pc["g0"]]

    last_run_info["maps"] = maps
    last_run_info["key"] = key
    return out


def measure_hw_ns(burst: int = 16):
    """On-device exec time via work-multiplying slope: amortized wall of a
    reps=R program minus the reps=1 program, divided by (R-1). Cancels all
    dispatch and transfer overheads; measures only the on-device time of one
    full pass (both GCN layers, AllGather exchange, pool + MLP head).
    Bursts of the two programs are interleaved and the median paired slope
    is reported to suppress drift on the shared device."""
    import time as _t
    key = last_run_info["key"]
    struct, percore, r1 = _CACHE[key]
    maps = last_run_info["maps"]
    R = 5

    runners = {}
    for reps in (1, R):
        ck = ("timing", reps)
        if ck not in _CACHE:
            nct = _build(struct, reps=reps)
            rx = _Runner(nct, donate=False)
            rx.stage(maps)
            _CACHE[ck] = rx
        runners[reps] = _CACHE[ck]
        runners[reps].run()  # warm

    def _burst(rx):
        t0 = _t.perf_counter()
        outs = None
        for _ in range(burst):
            outs = rx.fn(*rx._dev_in, *rx._zeros)
        for o in outs:
            o.block_until_ready()
        return (_t.perf_counter() - t0) / burst

    slopes = []
    detail = []
    for _ in range(7):
        t1 = _burst(runners[1])
        tR = _burst(runners[R])
        t1b = _burst(runners[1])
        slopes.append((tR - min(t1, t1b)) / (R - 1))
        detail.append((round(t1 * 1e6), round(tR * 1e6), round(t1b * 1e6)))
    slopes.sort()
    hw_s = slopes[len(slopes) // 2]
    last_run_info["hw_detail"] = {
        "pairs_us": detail,
        "slopes_us": [round(s * 1e6, 1) for s in slopes]}
    return max(hw_s, 0.0) * 1e9


# revision 9
# speedup vs baseline: 1.1077x; 1.0063x over previous
"""GCN graph classifier on 8 Trainium2 NeuronCores (Bass/Tile) — single launch.

v2 design (graph/data parallel per the sharding hint):
- Nodes split into 8 contiguous graph-aligned ranges; each core owns the
  destination side of every edge landing in its range, the pooling and the
  MLP head for its graphs.
- Both GCN layers, the inter-layer hidden-state exchange (DRAM AllGather),
  the mean-pool and the MLP head run in ONE device launch per kernel call.
- Tables live in DRAM in "slice layout" (row r2(n) = owner*W_SLOTS*128 +
  (n - n0[owner])), 256-byte bf16 rows; per-edge rows are fetched with
  dma_gather (int16 indices, 4 chunk windows of 32768 rows).
- Scatter-accumulate uses one-hot matmuls: per 128-edge block, a bf16
  one-hot built on DVE (iota == dst_rel, scaled by rsqrt(deg_src*deg_dst))
  feeds a PE matmul accumulating into a per-slot PSUM window. Blocks are
  packed without per-slot padding; a block whose edges straddle slot
  boundaries issues one matmul per touched slot.
- Self-loop terms use resident SBUF tiles (x / layer-1 hidden) with a
  diagonal one-hot of 1/deg, skipping ~25k gather descriptors per core.
- All per-core variation (indices, one-hot selectors, degrees) is input
  data; the compiled program is identical across cores (SPMD).

Self-contained: no imports from the problem directory.
"""
import time

import numpy as np

import concourse.bass as bass
import concourse.bacc as bacc
import concourse.mybir as mybir
import concourse.tile as tile

N_NODES = 100_000
N_EDGES = 1_200_000
N_GRAPHS = 512
HID = 64
NCORES = 8
P = 128
CHUNK_ROWS = 32_768
GSLOTS = 8                # slots per PSUM group
GMAX = 96                 # padded graphs per core
SENT = 30_000.0           # one-hot sentinel (bf16 -> 29952, never matches)

F32 = mybir.dt.float32
BF16 = mybir.dt.bfloat16
I16 = mybir.dt.int16
NPBF = mybir.dt.np(BF16)


# ---------------------------------------------------------------- host prep

def _prep(edge_index: np.ndarray, batch: np.ndarray):
    """Index-side preprocessing only: integer index manipulation derived from
    the graph structure plus integer degree counts (rsqrt happens on device)."""
    src = np.asarray(edge_index[0], dtype=np.int64)
    dst = np.asarray(edge_index[1], dtype=np.int64)
    batch = np.asarray(batch, dtype=np.int64)

    deg = np.bincount(dst, minlength=N_NODES) + 1  # +1 self-loop

    gptr = np.searchsorted(batch, np.arange(N_GRAPHS + 1))
    targets = (np.arange(1, NCORES) * N_NODES) // NCORES
    gsplit = np.searchsorted(gptr, targets)
    g0s = np.concatenate([[0], gsplit, [N_GRAPHS]])
    n0s = gptr[g0s]  # len 9
    nowns = np.diff(n0s)
    W_SLOTS = int(max(-(-n // P) for n in nowns))
    NTAB = NCORES * W_SLOTS * P
    NCHUNK = -(-NTAB // CHUNK_ROWS)
    NGRP = -(-W_SLOTS // GSLOTS)
    W_PAD = NGRP * GSLOTS
    assert max(g0s[c + 1] - g0s[c] for c in range(NCORES)) <= GMAX

    owner = np.searchsorted(n0s[1:], np.arange(N_NODES), side="right")
    r2 = owner * (W_SLOTS * P) + (np.arange(N_NODES) - n0s[owner])

    NCELL = NGRP * NCHUNK
    cores_raw = []
    cnt = np.zeros((NCORES, NCELL), dtype=np.int64)
    for c in range(NCORES):
        n0, n1 = int(n0s[c]), int(n0s[c + 1])
        m = (dst >= n0) & (dst < n1)
        es, ed = src[m], dst[m]
        slot = (ed - n0) >> 7
        grp = slot >> 3
        chunk = r2[es] >> 15
        o = np.lexsort((ed, slot, chunk, grp))
        es, ed, slot, grp, chunk = es[o], ed[o], slot[o], grp[o], chunk[o]
        cell = grp * NCHUNK + chunk
        cnt[c] = np.bincount(cell, minlength=NCELL)
        cores_raw.append((n0, n1, es, ed, slot, cell))

    K = -(-cnt.max(axis=0) // P)          # blocks per cell (0 stays 0)
    block_base = np.concatenate([[0], np.cumsum(K)[:-1]])
    NBLK = int(K.sum())
    NE = NBLK * P

    # calls: one gather per non-empty (group, chunk)
    calls = []  # (g, ch, blk0, nblk)
    for g in range(NGRP):
        for ch in range(NCHUNK):
            cell = g * NCHUNK + ch
            if K[cell] > 0:
                calls.append((g, ch, int(block_base[cell]), int(K[cell])))
    call_of_cell = {}
    for ci, (g, ch, b0, nb) in enumerate(calls):
        call_of_cell[g * NCHUNK + ch] = ci

    # per-core edge placement + per-block slot spans
    lo = np.full(NBLK, 1 << 30, dtype=np.int64)
    hi = np.full(NBLK, -1, dtype=np.int64)
    percore_edges = []
    for c in range(NCORES):
        n0, n1, es, ed, slot, cell = cores_raw[c]
        cell_start = np.concatenate([[0], np.cumsum(np.bincount(
            cell, minlength=NCELL))])[:-1]
        rank = np.arange(len(es)) - cell_start[cell]
        blk = block_base[cell] + (rank >> 7)
        pos = blk * P + (rank & 127)
        np.minimum.at(lo, blk, slot)
        np.maximum.at(hi, blk, slot)
        percore_edges.append((n0, es, ed, slot, blk, rank & 127))
    assert (hi >= 0).all()

    # mm schedule (shared across cores). PSUM accumulation groups to the same
    # bank must be contiguous in PE issue order, so emit slot-major: each
    # slot's self mm + edge mms form one contiguous start..stop group.
    mm_list = []   # (kind, g, s, blk, ci, kloc, col) col=dst_rel column
    NMME = 0
    for g in range(NGRP):
        per_slot = {s: [] for s in range(g * GSLOTS, (g + 1) * GSLOTS)}
        for ch in range(NCHUNK):
            cell = g * NCHUNK + ch
            for k in range(int(K[cell])):
                blk = int(block_base[cell]) + k
                for s in range(int(lo[blk]), int(hi[blk]) + 1):
                    per_slot[s].append((blk, call_of_cell[cell], k))
        for s in range(g * GSLOTS, (g + 1) * GSLOTS):
            mm_list.append(["self", g, s, -1, -1, -1, -1])
            for blk, ci, k in per_slot[s]:
                mm_list.append(["edge", g, s, blk, ci, k, NMME])
                NMME += 1
            mm_list[-1].append("stop")
    NMM = len(mm_list)

    # per-core data arrays
    rng_pad = np.random.default_rng(12345)
    percore = []
    for c in range(NCORES):
        n0, es, ed, slot, blk, wpos = percore_edges[c]

        idx_flat = rng_pad.integers(0, 2048, size=NE).astype(np.int16)
        idx_flat[blk * P + wpos] = (r2[es] & (CHUNK_ROWS - 1)).astype(np.int16)
        cols = NE // 16
        arr = np.zeros((16, cols), dtype=np.int16)
        j = np.arange(NE)
        arr[j % 16, j // 16] = idx_flat
        idx_packed = np.tile(arr, (8, 1))

        # dst_rel: one column per edge mm
        mme_of_blk_s = {}
        for mi, mm in enumerate(mm_list):
            if mm[0] == "edge":
                mme_of_blk_s[(mm[3], mm[2])] = mm[6]
        col = np.array([mme_of_blk_s[(b, s)] for b, s in zip(blk, slot)])
        dst_rel = np.full((P, NMME), SENT, dtype=np.float32)
        dst_rel[wpos, col] = (ed - n0 - slot * P).astype(np.float32)

        dsnp = np.ones((P, NBLK), dtype=np.float32)
        dsnp[wpos, blk] = (deg[es] * deg[ed]).astype(np.float32)

        nown = int(n0s[c + 1]) - n0
        ar = np.arange(nown)
        deg_own = np.ones((P, W_PAD), dtype=np.float32)
        deg_own[ar % P, ar >> 7] = deg[n0:n0 + nown].astype(np.float32)
        g_rel = np.full((P, W_PAD), SENT, dtype=np.float32)
        g_rel[ar % P, ar >> 7] = (batch[n0:n0 + nown] - g0s[c]).astype(
            np.float32)

        percore.append({
            "n0": n0, "nown": nown, "g0": int(g0s[c]), "g1": int(g0s[c + 1]),
            "idx_packed": idx_packed,
            "dst_rel": dst_rel,
            "dsnp": dsnp,
            "deg_own": deg_own,
            "g_rel": g_rel,
        })

    struct = {
        "W_SLOTS": W_SLOTS, "W_PAD": W_PAD, "NTAB": NTAB, "NGRP": NGRP,
        "NCHUNK": NCHUNK, "NBLK": NBLK, "NE": NE, "NMM": NMM, "NMME": NMME,
        "calls": calls, "mm_list": mm_list, "r2": r2, "n0s": n0s, "g0s": g0s,
    }
    return struct, percore


# ------------------------------------------------------------- bass program

def _build(struct, reps: int = 1, dump: bool = False,
           coll_reps: bool = True, l2_reps: bool = True, part: str = 'all'):
    W_SLOTS, W_PAD = struct["W_SLOTS"], struct["W_PAD"]
    NTAB, NGRP, NCHUNK = struct["NTAB"], struct["NGRP"], struct["NCHUNK"]
    NBLK, NE, NMME = struct["NBLK"], struct["NE"], struct["NMME"]
    calls, mm_list = struct["calls"], struct["mm_list"]

    nc = bacc.Bacc("TRN2", num_swdge_queues=4,
                   dynamic_dma_scratch_size=49152)
    xtab = nc.dram_tensor("xtab", (NTAB, P), BF16, kind="ExternalInput")
    idx_in = nc.dram_tensor("idx", (P, NE // 16), I16, kind="ExternalInput")
    dst_rel_in = nc.dram_tensor("dst_rel", (P, NMME), F32,
                                kind="ExternalInput")
    dsnp_in = nc.dram_tensor("dsnp", (P, NBLK), F32, kind="ExternalInput")
    deg_own_in = nc.dram_tensor("deg_own", (P, W_PAD), F32,
                                kind="ExternalInput")
    g_rel_in = nc.dram_tensor("g_rel", (P, W_PAD), F32, kind="ExternalInput")
    x_own_in = nc.dram_tensor("x_own", (P, 2 * W_PAD), BF16,
                              kind="ExternalInput")
    iota_in = nc.dram_tensor("iota", (P, P), BF16, kind="ExternalInput")
    self_rel_in = nc.dram_tensor("self_rel", (P, 1), F32,
                                 kind="ExternalInput")
    ident_in = nc.dram_tensor("ident", (P, P), F32, kind="ExternalInput")
    ident_bf_in = nc.dram_tensor("ident_bf", (P, P), BF16,
                                 kind="ExternalInput")
    ones_bf_in = nc.dram_tensor("ones_bf", (1, P), BF16, kind="ExternalInput")
    ones_f_in = nc.dram_tensor("ones_f", (1, P), F32, kind="ExternalInput")
    w1_in = nc.dram_tensor("W1", (2, HID), BF16, kind="ExternalInput")
    b1_in = nc.dram_tensor("b1", (1, HID), BF16, kind="ExternalInput")
    w2_in = nc.dram_tensor("W2", (HID, HID), BF16, kind="ExternalInput")
    b2_in = nc.dram_tensor("b2", (1, HID), BF16, kind="ExternalInput")
    wf1_in = nc.dram_tensor("Wf1", (HID, HID), F32, kind="ExternalInput")
    bf1_in = nc.dram_tensor("bf1", (1, HID), F32, kind="ExternalInput")
    wf2_in = nc.dram_tensor("Wf2", (HID, 4), F32, kind="ExternalInput")
    bf2_in = nc.dram_tensor("bf2", (1, 4), F32, kind="ExternalInput")

    h1slice = nc.dram_tensor("h1slice", (W_SLOTS * P, P), BF16)
    if dump:
        h1dump = nc.dram_tensor("h1dump", (W_SLOTS * P, HID), BF16,
                                kind="ExternalOutput")
    h1tab = nc.dram_tensor("h1tab", (NTAB, P), BF16)
    out_t = nc.dram_tensor("out", (GMAX, 4), F32, kind="ExternalOutput")

    with tile.TileContext(nc) as tc:
        with tc.tile_pool(name="const", bufs=1) as cpool, \
             tc.tile_pool(name="meta", bufs=1) as mpool, \
             tc.tile_pool(name="gat", bufs=3) as gpool, \
             tc.tile_pool(name="work", bufs=3) as wpool, \
             tc.tile_pool(name="oh", bufs=48) as ohpool, \
             tc.tile_pool(name="pacc", bufs=3, space="PSUM") as pacc, \
             tc.tile_pool(name="ptp", bufs=2, space="PSUM") as ptp, \
             tc.tile_pool(name="ppool", bufs=1, space="PSUM") as ppool:

            # ---- constants / metadata
            iota_t = cpool.tile([P, P], BF16)
            nc.sync.dma_start(out=iota_t[:], in_=iota_in[:])
            self_rel_t = cpool.tile([P, 1], F32)
            nc.sync.dma_start(out=self_rel_t[:], in_=self_rel_in[:])
            ident_t = cpool.tile([P, P], F32)
            nc.sync.dma_start(out=ident_t[:], in_=ident_in[:])
            ident_bf_t = cpool.tile([P, P], BF16)
            nc.sync.dma_start(out=ident_bf_t[:], in_=ident_bf_in[:])
            ones_bf_t = cpool.tile([1, P], BF16)
            nc.sync.dma_start(out=ones_bf_t[:], in_=ones_bf_in[:])
            ones_f_t = cpool.tile([1, P], F32)
            nc.sync.dma_start(out=ones_f_t[:], in_=ones_f_in[:])
            onecol_t = cpool.tile([P, 1], F32)
            nc.vector.memset(onecol_t[:], 1.0)
            w1_t = cpool.tile([2, HID], BF16)
            nc.sync.dma_start(out=w1_t[:], in_=w1_in[:])
            b1_t = cpool.tile([1, HID], BF16)
            nc.sync.dma_start(out=b1_t[:], in_=b1_in[:])
            w2_t = cpool.tile([HID, HID], BF16)
            nc.sync.dma_start(out=w2_t[:], in_=w2_in[:])
            b2_t = cpool.tile([1, HID], BF16)
            nc.sync.dma_start(out=b2_t[:], in_=b2_in[:])
            wf1_t = cpool.tile([HID, HID], F32)
            nc.sync.dma_start(out=wf1_t[:], in_=wf1_in[:])
            bf1_t = cpool.tile([1, HID], F32)
            nc.sync.dma_start(out=bf1_t[:], in_=bf1_in[:])
            wf2_t = cpool.tile([HID, 4], F32)
            nc.sync.dma_start(out=wf2_t[:], in_=wf2_in[:])
            bf2_t = cpool.tile([1, 4], F32)
            nc.sync.dma_start(out=bf2_t[:], in_=bf2_in[:])

            idx_t = mpool.tile([P, NE // 16], I16)
            nc.sync.dma_start(out=idx_t[:], in_=idx_in[:])
            dst_rel_t = mpool.tile([P, NMME], F32)
            nc.sync.dma_start(out=dst_rel_t[:], in_=dst_rel_in[:])
            g_rel_t = mpool.tile([P, W_PAD], F32)
            nc.sync.dma_start(out=g_rel_t[:], in_=g_rel_in[:])
            x_own_t = mpool.tile([P, 2 * W_PAD], BF16)
            nc.sync.dma_start(out=x_own_t[:], in_=x_own_in[:])

            # dsn = rsqrt(deg_src*deg_dst) in bf16
            dsnp_t = mpool.tile([P, NBLK], F32)
            nc.sync.dma_start(out=dsnp_t[:], in_=dsnp_in[:])
            dsnr_t = mpool.tile([P, NBLK], F32)
            nc.vector.reciprocal(out=dsnr_t[:], in_=dsnp_t[:])
            dsn_t = mpool.tile([P, NBLK], F32)
            nc.scalar.sqrt(out=dsn_t[:], in_=dsnr_t[:])

            # dinv2_own = 1/deg_own in bf16
            deg_own_t = mpool.tile([P, W_PAD], F32)
            nc.sync.dma_start(out=deg_own_t[:], in_=deg_own_in[:])
            dinv2_t = mpool.tile([P, W_PAD], F32)
            nc.vector.reciprocal(out=dinv2_t[:], in_=deg_own_t[:])

            # resident layer-1 hidden (own nodes), persists across layers
            h1own_t = mpool.tile([P, W_PAD * HID], BF16)

            dumtiles = None
            if part in ("mm", "ohonly", "tails"):
                maxnb = max(nb for (_, _, _, nb) in calls)
                dumtiles = {}
                for ch in range(NCHUNK):
                    dt_ = mpool.tile([P, maxnb, P], BF16, tag=f"dum{ch}")
                    nc.vector.memset(dt_[:], 0.125)
                    dumtiles[ch] = dt_

            pool_ps = ppool.tile([GMAX, HID + 1], F32, space="PSUM")

            for rep in range(reps):
                for layer in ((1, 2) if (rep == 0 or l2_reps) else (1,)):
                    F = 2 if layer == 1 else HID
                    w_t = w1_t if layer == 1 else w2_t
                    b_t = b1_t if layer == 1 else b2_t
                    full = rep == 0 or part == "all"

                    def tails(g, acc):
                        # per-slot tails (deferred one group for pipelining)
                        a2g = wpool.tile([P, GSLOTS * HID], BF16, tag="a2")
                        nc.scalar.mul(a2g[:], acc[:], onecol_t[:])
                        for sloc in range(GSLOTS):
                            s = g * GSLOTS + sloc
                            tp = ptp.tile([F, P], BF16, space="PSUM",
                                          tag="tp")
                            nc.tensor.transpose(
                                out=tp[:],
                                in_=a2g[:, sloc * HID:sloc * HID + F],
                                identity=ident_bf_t[:])
                            a2t = wpool.tile([F, P], BF16, tag="a2t")
                            nc.scalar.mul(a2t[:], tp[:], onecol_t[0:F, :])
                            hps = ptp.tile([P, HID], F32, space="PSUM",
                                           tag="hps")
                            nc.tensor.matmul(out=hps[:], lhsT=ones_bf_t[:],
                                             rhs=b_t[:], start=True,
                                             stop=False)
                            nc.tensor.matmul(out=hps[:], lhsT=a2t[:],
                                             rhs=w_t[:], start=False,
                                             stop=True)
                            if layer == 1:
                                nc.scalar.activation(
                                    out=h1own_t[:, HID * s:HID * (s + 1)],
                                    in_=hps[:],
                                    func=mybir.ActivationFunctionType.Relu)
                                if s < W_SLOTS:
                                    nc.sync.dma_start(
                                        out=h1slice[s * P:(s + 1) * P, 0:HID],
                                        in_=h1own_t[:, HID * s:HID * (s + 1)])
                                    if dump:
                                        nc.sync.dma_start(
                                            out=h1dump[s * P:(s + 1) * P, :],
                                            in_=h1own_t[:,
                                                        HID * s:HID * (s + 1)])
                            else:
                                h2e = wpool.tile([P, HID + 1], BF16,
                                                 tag="h2e")
                                nc.scalar.activation(
                                    out=h2e[:, 0:HID], in_=hps[:],
                                    func=mybir.ActivationFunctionType.Relu)
                                nc.vector.memset(h2e[:, HID:HID + 1], 1.0)
                                goh = ohpool.tile([P, GMAX], BF16, tag="goh")
                                nc.vector.tensor_scalar(
                                    out=goh[:], in0=iota_t[:, 0:GMAX],
                                    scalar1=g_rel_t[:, s:s + 1], scalar2=None,
                                    op0=mybir.AluOpType.is_equal)
                                nc.tensor.matmul(out=pool_ps[:], lhsT=goh[:],
                                                 rhs=h2e[:], start=(s == 0),
                                                 stop=(s == W_PAD - 1))

                    pending = None
                    for g in range(NGRP):
                        gtiles = {}
                        for (cg, ch, blk0, nb) in calls:
                            if cg != g:
                                continue
                            g_t = gpool.tile([P, nb, P], BF16, tag=f"g{ch}")
                            table = xtab if layer == 1 else h1tab
                            r0 = ch * CHUNK_ROWS
                            r1 = min(r0 + CHUNK_ROWS, NTAB)
                            if full or part == "gather":
                                nc.gpsimd.dma_gather(
                                    out_ap=g_t[:],
                                    in_ap=table[r0:r1, :],
                                    idxs_ap=idx_t[:, blk0 * 8:(blk0 + nb) * 8],
                                    num_idxs=nb * P, num_idxs_reg=nb * P,
                                    elem_size=P, single_packet=False,
                                    queue_num=ch)
                                gtiles[ch] = g_t
                            else:
                                gtiles[ch] = dumtiles[ch]
                        if not full and part == "gather":
                            continue
                        if not full and part in ("mm", "ohonly", "tails"):
                            pass

                        acc = pacc.tile([P, GSLOTS * HID], F32, space="PSUM",
                                        tag="acc")
                        mms = [m for m in mm_list if m[1] == g]
                        if not full and part == "tails":
                            mms = [m for m in mms if m[0] == "self"]
                        for mm in mms:
                            kind, _, s, blk, ci, kloc = mm[:6]
                            sloc = s - g * GSLOTS
                            stop = mm[-1] == "stop"
                            oh = ohpool.tile([P, P], BF16, tag="oh")
                            if kind == "self":
                                nc.vector.tensor_scalar(
                                    out=oh[:], in0=iota_t[:],
                                    scalar1=self_rel_t[:],
                                    scalar2=dinv2_t[:, s:s + 1],
                                    op0=mybir.AluOpType.is_equal,
                                    op1=mybir.AluOpType.mult)
                                if layer == 1:
                                    rhs = x_own_t[:, 2 * s:2 * s + 2]
                                else:
                                    rhs = h1own_t[:, HID * s:HID * (s + 1)]
                                nc.tensor.matmul(
                                    out=acc[:, sloc * HID:sloc * HID + F],
                                    lhsT=oh[:], rhs=rhs,
                                    start=True,
                                    stop=stop or (not full and
                                                  part == "tails"))
                            else:
                                col = mm[6]
                                call_ch = calls[ci][1]
                                nc.vector.tensor_scalar(
                                    out=oh[:], in0=iota_t[:],
                                    scalar1=dst_rel_t[:, col:col + 1],
                                    scalar2=dsn_t[:, blk:blk + 1],
                                    op0=mybir.AluOpType.is_equal,
                                    op1=mybir.AluOpType.mult)
                                if full or part == "mm":
                                    nc.tensor.matmul(
                                        out=acc[:, sloc * HID:sloc * HID + F],
                                        lhsT=oh[:],
                                        rhs=gtiles[call_ch][:, kloc, 0:F],
                                        start=False, stop=stop)

                        # ---- tails, deferred one group
                        if not full and part in ("mm", "ohonly"):
                            continue
                        if pending is not None:
                            tails(*pending)
                        pending = (g, acc)
                    if pending is not None:
                        tails(*pending)

                    if layer == 1 and (rep == 0 or coll_reps):
                        nc.gpsimd.collective_compute(
                            kind="AllGather",
                            op=mybir.AluOpType.bypass,
                            replica_groups=[list(range(NCORES))],
                            ins=[h1slice[:]],
                            outs=[h1tab[:]],
                        )

                # ---- mean-pool + MLP head
                pool_sb = wpool.tile([GMAX, HID + 1], F32, tag="pool")
                nc.vector.tensor_copy(out=pool_sb[:], in_=pool_ps[:])
                cntm = wpool.tile([GMAX, 1], F32, tag="cnt")
                nc.vector.tensor_scalar(
                    out=cntm[:], in0=pool_sb[:, HID:HID + 1], scalar1=1.0,
                    scalar2=None, op0=mybir.AluOpType.max)
                rcnt = wpool.tile([GMAX, 1], F32, tag="rcnt")
                nc.vector.reciprocal(out=rcnt[:], in_=cntm[:])
                means = wpool.tile([GMAX, HID], F32, tag="means")
                nc.scalar.mul(means[:], pool_sb[:, 0:HID], rcnt[:])
                mt_ps = ptp.tile([HID, GMAX], F32, space="PSUM", tag="tp")
                nc.tensor.transpose(out=mt_ps[:], in_=means[:],
                                    identity=ident_t[0:GMAX, 0:GMAX])
                mt = wpool.tile([HID, GMAX], F32, tag="mt")
                nc.vector.tensor_copy(out=mt[:], in_=mt_ps[:])
                f1_ps = ptp.tile([GMAX, HID], F32, space="PSUM", tag="hps")
                nc.tensor.matmul(out=f1_ps[:], lhsT=ones_f_t[:, 0:GMAX],
                                 rhs=bf1_t[:], start=True, stop=False)
                nc.tensor.matmul(out=f1_ps[:], lhsT=mt[:], rhs=wf1_t[:],
                                 start=False, stop=True)
                f1 = wpool.tile([GMAX, HID], F32, tag="f1")
                nc.scalar.activation(out=f1[:], in_=f1_ps[:],
                                     func=mybir.ActivationFunctionType.Relu)
                f1t_ps = ptp.tile([HID, GMAX], F32, space="PSUM", tag="tp")
                nc.tensor.transpose(out=f1t_ps[:], in_=f1[:],
                                    identity=ident_t[0:GMAX, 0:GMAX])
                f1t = wpool.tile([HID, GMAX], F32, tag="f1t")
                nc.vector.tensor_copy(out=f1t[:], in_=f1t_ps[:])
                o_ps = ptp.tile([GMAX, 4], F32, space="PSUM", tag="hps")
                nc.tensor.matmul(out=o_ps[:], lhsT=ones_f_t[:, 0:GMAX],
                                 rhs=bf2_t[:], start=True, stop=False)
                nc.tensor.matmul(out=o_ps[:], lhsT=f1t[:], rhs=wf2_t[:],
                                 start=False, stop=True)
                o_sb = wpool.tile([GMAX, 4], F32, tag="osb")
                nc.vector.tensor_copy(out=o_sb[:], in_=o_ps[:])
                nc.sync.dma_start(out=out_t[:], in_=o_sb[:])

    nc.finalize()
    return nc


# ---------------------------------------------------------------- pjrt run

class _Runner:
    def __init__(self, nc, n_cores: int = NCORES, donate: bool = True):
        import jax
        from jax.sharding import Mesh, NamedSharding, PartitionSpec
        from jax.experimental.shard_map import shard_map
        from concourse.bass2jax import (
            _bass_exec_p, install_neuronx_cc_hook, partition_id_tensor)

        install_neuronx_cc_hook()
        self.jax = jax
        self.n_cores = n_cores
        in_names, out_names, out_avals = [], [], []
        pname = nc.partition_id_tensor.name if nc.partition_id_tensor else None
        for alloc in nc.m.functions[0].allocations:
            if not isinstance(alloc, mybir.MemoryLocationSet):
                continue
            name = alloc.memorylocations[0].name
            if alloc.kind == "ExternalInput":
                if name != pname:
                    in_names.append(name)
            elif alloc.kind == "ExternalOutput":
                out_names.append(name)
                out_avals.append(jax.core.ShapedArray(
                    tuple(alloc.tensor_shape), mybir.dt.np(alloc.dtype)))
        self.in_names, self.out_names, self.out_avals = (
            in_names, out_names, out_avals)
        n_params, n_outs = len(in_names), len(out_avals)
        all_in = in_names + out_names + ([pname] if pname else [])

        def _body(*args):
            operands = list(args)
            if pname:
                operands.append(partition_id_tensor())
            return tuple(_bass_exec_p.bind(
                *operands, out_avals=tuple(out_avals),
                in_names=tuple(all_in), out_names=tuple(out_names),
                lowering_input_output_aliases=(),
                sim_require_finite=False, sim_require_nnan=False, nc=nc))

        devices = jax.devices()[:n_cores]
        self.mesh = Mesh(np.asarray(devices), ("core",))
        self.sh = NamedSharding(self.mesh, PartitionSpec("core"))
        smapped = shard_map(
            _body, mesh=self.mesh,
            in_specs=(PartitionSpec("core"),) * (n_params + n_outs),
            out_specs=(PartitionSpec("core"),) * n_outs,
            check_rep=False)
        if donate:
            self.fn = jax.jit(
                smapped,
                donate_argnums=tuple(range(n_params, n_params + n_outs)),
                keep_unused=True)
        else:
            self.fn = jax.jit(smapped, keep_unused=True)
        self.donate = donate
        self._zs = [(n_cores * a.shape[0], *a.shape[1:]) for a in out_avals]
        self._zd = [a.dtype for a in out_avals]
        self._dev_in = None
        self._zeros = None

    def stage(self, in_maps):
        ci = [np.concatenate([np.ascontiguousarray(in_maps[c][n])
                              for c in range(self.n_cores)], axis=0)
              for n in self.in_names]
        self._dev_in = [self.jax.device_put(x, self.sh) for x in ci]
        for x in self._dev_in:
            x.block_until_ready()
        if not self.donate:
            self._zeros = [self.jax.device_put(np.zeros(s, d), self.sh)
                           for s, d in zip(self._zs, self._zd)]
            for z in self._zeros:
                z.block_until_ready()

    def run(self):
        if self.donate:
            zeros = [self.jax.device_put(np.zeros(s, d), self.sh)
                     for s, d in zip(self._zs, self._zd)]
        else:
            zeros = self._zeros
        outs = self.fn(*self._dev_in, *zeros)
        for o in outs:
            o.block_until_ready()
        return outs

    def results(self, outs):
        res = []
        for c in range(self.n_cores):
            d = {}
            for i, n in enumerate(self.out_names):
                a = np.asarray(outs[i]).reshape(
                    self.n_cores, *self.out_avals[i].shape)
                d[n] = a[c]
            res.append(d)
        return res


# ----------------------------------------------------------------- kernel()

_CACHE = {}
last_run_info = {}


def _consts():
    iota = np.tile(np.arange(P, dtype=np.float32), (P, 1)).astype(NPBF)
    ident_bf = np.eye(P, dtype=np.float32).astype(NPBF)
    self_rel = np.arange(P, dtype=np.float32).reshape(P, 1)
    ident = np.eye(P, dtype=np.float32)
    ones_bf = np.ones((1, P), NPBF)
    ones_f = np.ones((1, P), np.float32)
    return iota, ident_bf, self_rel, ident, ones_bf, ones_f


def _maps(struct, percore, x, W1, b1, W2, b2, Wf1, bf1, Wf2, bf2):
    iota, ident_bf, self_rel, ident, ones_bf, ones_f = _consts()
    NTAB, W_PAD = struct["NTAB"], struct["W_PAD"]
    r2, n0s = struct["r2"], struct["n0s"]

    xtab = np.zeros((NTAB, P), NPBF)
    xtab[r2, 0:2] = x.astype(NPBF)

    maps = []
    for c in range(NCORES):
        pc = percore[c]
        n0, nown = pc["n0"], pc["nown"]
        x_own = np.zeros((P, 2 * W_PAD), NPBF)
        ar = np.arange(nown)
        x_own[ar % P, 2 * (ar >> 7)] = x[n0:n0 + nown, 0].astype(NPBF)
        x_own[ar % P, 2 * (ar >> 7) + 1] = x[n0:n0 + nown, 1].astype(NPBF)
        maps.append({
            "xtab": xtab, "idx": pc["idx_packed"], "dst_rel": pc["dst_rel"],
            "dsnp": pc["dsnp"], "deg_own": pc["deg_own"],
            "g_rel": pc["g_rel"], "x_own": x_own,
            "iota": iota, "ident_bf": ident_bf,
            "self_rel": self_rel, "ident": ident,
            "ones_bf": ones_bf, "ones_f": ones_f,
            "W1": W1.astype(NPBF), "b1": b1.astype(NPBF),
            "W2": W2.astype(NPBF), "b2": b2.astype(NPBF),
            "Wf1": Wf1, "bf1": bf1, "Wf2": Wf2, "bf2": bf2,
        })
    return maps


def kernel(x, edge_index, batch, num_graphs=None, W1=None, b1=None, W2=None,
           b2=None, Wf1=None, bf1=None, Wf2=None, bf2=None):
    x = np.asarray(x, dtype=np.float32)
    W1 = np.asarray(W1, dtype=np.float32)
    b1 = np.asarray(b1, dtype=np.float32).reshape(1, HID)
    W2 = np.asarray(W2, dtype=np.float32)
    b2 = np.asarray(b2, dtype=np.float32).reshape(1, HID)
    Wf1 = np.asarray(Wf1, dtype=np.float32)
    bf1 = np.asarray(bf1, dtype=np.float32).reshape(1, HID)
    Wf2 = np.asarray(Wf2, dtype=np.float32)
    bf2 = np.asarray(bf2, dtype=np.float32).reshape(1, 4)

    ei = np.asarray(edge_index)
    bt = np.asarray(batch)
    key = hash((ei.tobytes(), bt.tobytes()))
    if key not in _CACHE:
        t0 = time.time()
        struct, percore = _prep(ei, bt)
        nc1 = _build(struct, reps=1)
        r1 = _Runner(nc1)
        _CACHE[key] = (struct, percore, r1)
        last_run_info["build_s"] = time.time() - t0
    struct, percore, r1 = _CACHE[key]

    maps = _maps(struct, percore, x, W1, b1, W2, b2, Wf1, bf1, Wf2, bf2)
    t0 = time.time()
    r1.stage(maps)
    last_run_info["stage_s"] = time.time() - t0
    t0 = time.time()
    outs = r1.run()
    last_run_info["run_s"] = time.time() - t0
    res = r1.results(outs)

    out = np.zeros((N_GRAPHS, 4), dtype=np.float32)
    for c in range(NCORES):
        pc = percore[c]
        out[pc["g0"]:pc["g1"]] = res[c]["out"][0:pc["g1"] - pc["g0"]]

    last_run_info["maps"] = maps
    last_run_info["key"] = key
    return out


def measure_hw_ns(burst: int = 16):
    """On-device exec time via work-multiplying slope: amortized wall of a
    reps=R program minus the reps=1 program, divided by (R-1). Cancels all
    dispatch and transfer overheads; measures only the on-device time of one
    full pass (both GCN layers, AllGather exchange, pool + MLP head).
    Bursts of the two programs are interleaved and the median paired slope
    is reported to suppress drift on the shared device."""
    import time as _t
    key = last_run_info["key"]
    struct, percore, r1 = _CACHE[key]
    maps = last_run_info["maps"]
    R = 5

    runners = {}
    for reps in (1, R):
        ck = ("timing", reps)
        if ck not in _CACHE:
            nct = _build(struct, reps=reps)
            rx = _Runner(nct, donate=False)
            rx.stage(maps)
            _CACHE[ck] = rx
        runners[reps] = _CACHE[ck]
        runners[reps].run()  # warm

    def _burst(rx):
        t0 = _t.perf_counter()
        outs = None
        for _ in range(burst):
            outs = rx.fn(*rx._dev_in, *rx._zeros)
        for o in outs:
            o.block_until_ready()
        return (_t.perf_counter() - t0) / burst

    slopes = []
    detail = []
    for _ in range(7):
        t1 = _burst(runners[1])
        tR = _burst(runners[R])
        t1b = _burst(runners[1])
        slopes.append((tR - min(t1, t1b)) / (R - 1))
        detail.append((round(t1 * 1e6), round(tR * 1e6), round(t1b * 1e6)))
    slopes.sort()
    hw_s = slopes[len(slopes) // 2]
    last_run_info["hw_detail"] = {
        "pairs_us": detail,
        "slopes_us": [round(s * 1e6, 1) for s in slopes]}
    return max(hw_s, 0.0) * 1e9
